# revision 21
# baseline (speedup 1.0000x reference)
"""Trainium2 Bass kernel for nn_ColorExtractor (per-image k-means, K=8, 10 iters).

Contract: kernel(**inputs) takes FULL inputs ([64, 512, 512, 3] f32), returns
FULL output ([64, 24] f32), batch sharded over 8 NeuronCores (8 images/core).

v3 design (vs v2):
  seg     segment-sum matmuls run in fp8 DoubleRow mode at 2x PE rate: the
          fp16 0/1 mask buffer is bitcast to fp8e4 pairs (lo byte always
          0x00 -> 0.0, hi byte 0x3C -> 1.5) and fed as the two DoubleRow
          contraction subtiles; xpix is stored fp8 with a zeroed partner
          subtile. Sums come out scaled by exactly 1.5, which cancels in
          means; the count threshold drops to 0.5 (n=0 -> masked anyway).
  argmin  DVE work batched at 32-group granularity (half the instruction
          dispatch of v2); min tree + is_le unchanged otherwise.
  weights w8 = [-2c | sum c^2] built on ACT (mul + Square-with-accum),
          freeing DVE cycles.

Initial centroids replicate jax.random.permutation(key, N)[:8] via the
precomputed PERM8 table (numpy threefry port, verified bit-exact).
"""

import numpy as np
import ml_dtypes

import concourse.bacc as bacc
import concourse.bass as bass
import concourse.tile as tile
from concourse import mybir
from concourse.bass_utils import run_bass_kernel_spmd

# ----------------------------------------------------------------------------
# problem constants (hardcoded per contract)
B = 64            # total images
NCORES = 8
IMG_PER_CORE = B // NCORES
H = W = 512
N = H * W         # pixels per image: 262144
K = 8             # clusters
ITERS = 10
D = 3

# device tiling
P = 128           # pixels per chunk
J = 16            # chunks per matmul group (block-diag j packing)
NB = 8            # contraction bands: r,g,b, r,g,b(lo), ones, |x|^2
GROUPS = N // (J * P)    # 128 groups per image
GBATCH = 8        # groups per PSUM batch
SDB = 4 * GBATCH  # groups per DVE super-batch (4 PSUM fills)
NSUP = GROUPS // SDB     # 4
FD = K * J        # 128: free dim of scores/seg matmuls, cols = (k, j)
CJ = 4 * J        # 64: xpix cols per group, (c in {r,g,b,1}, j)

F32 = mybir.dt.float32
F16 = mybir.dt.float16
F8 = mybir.dt.float8e4

# fp8 DoubleRow segment sums (mask byte-alias trick): measured rel err
# 1.98e-2 vs the 2e-2 gate (fp8 xpix quantization excites trajectory chaos
# in k-means) and the DR matmuls serialize LDWEIGHTS. Keep the fp16 path.
SEG_FP8 = False

# ----------------------------------------------------------------------------
# numpy threefry port (verified bit-exact vs jax 0.8 threefry2x32 impl)
_U32 = np.uint32


def _rotl(x, d):
    d = _U32(d)
    return (x << d) | (x >> _U32(32 - d))


def _threefry2x32(k1, k2, x1, x2):
    with np.errstate(over="ignore"):
        ks0, ks1 = _U32(k1), _U32(k2)
        ks2 = _U32(ks0 ^ ks1 ^ _U32(0x1BD11BDA))
        x = [(x1 + ks0).astype(_U32), (x2 + ks1).astype(_U32)]

        def rounds(rots, ka, kb, inc):
            for r in rots:
                x[0] = (x[0] + x[1]).astype(_U32)
                x[1] = _rotl(x[1], r)
                x[1] = x[0] ^ x[1]
            x[0] = (x[0] + ka).astype(_U32)
            x[1] = (x[1] + kb + _U32(inc)).astype(_U32)

        rounds((13, 15, 26, 6), ks1, ks2, 1)
        rounds((17, 29, 16, 24), ks2, ks0, 2)
        rounds((13, 15, 26, 6), ks0, ks1, 3)
        rounds((17, 29, 16, 24), ks1, ks2, 4)
        rounds((13, 15, 26, 6), ks2, ks0, 5)
    return x[0], x[1]


def _tf_split(key, num):
    i = np.arange(num, dtype=np.uint64)
    b1, b2 = _threefry2x32(key[0], key[1],
                           (i >> np.uint64(32)).astype(_U32), i.astype(_U32))
    return np.stack([b1, b2], axis=1)


def _tf_bits(key, n):
    i = np.arange(n, dtype=np.uint64)
    b1, b2 = _threefry2x32(key[0], key[1],
                           (i >> np.uint64(32)).astype(_U32), i.astype(_U32))
    return b1 ^ b2


def jax_permutation_indices(seed, batch, n):
    """perm[b] = jax.random.permutation(split(key(seed), batch)[b], n)."""
    keys = _tf_split(np.array([0, seed], _U32), batch)
    num_rounds = int(np.ceil(3 * np.log(max(1, n)) / np.log(2**32 - 1)))
    perms = []
    for b in range(batch):
        x = np.arange(n)
        k = keys[b]
        for _ in range(num_rounds):
            ks = _tf_split(k, 2)
            k = ks[0]
            sort_keys = _tf_bits(ks[1], n)
            x = x[np.argsort(sort_keys, kind="stable")]
        perms.append(x[:K])
    return np.stack(perms)  # [batch, K]


# Precomputed jax.random.permutation(split(key(42), 64)[b], N)[:8] indices
# (input-independent; verified against the threefry port above).
PERM8 = (
    (121373, 128858, 64733, 199519, 198377, 234239, 198325, 209106),
    (73520, 236184, 209288, 97370, 64322, 228694, 126128, 72161),
    (143944, 27877, 97040, 2149, 10994, 109181, 179954, 54887),
    (147613, 8773, 54262, 44295, 29289, 11407, 31612, 133442),
    (206432, 166428, 5023, 212109, 16365, 21194, 249053, 195143),
    (13257, 110295, 84080, 119151, 246640, 69532, 130091, 105945),
    (14760, 174397, 198857, 826, 140745, 258776, 214608, 163989),
    (184593, 240934, 160738, 23779, 43199, 47433, 94941, 50416),
    (4386, 21260, 129661, 125128, 50701, 200388, 254109, 44816),
    (203980, 230711, 102351, 31296, 161690, 63692, 194032, 60281),
    (170168, 75997, 12072, 137876, 34146, 48636, 181597, 67859),
    (218987, 48148, 224774, 27163, 85280, 163529, 107708, 238871),
    (152153, 120028, 50368, 168498, 254864, 185234, 259971, 5221),
    (126051, 57270, 7614, 194865, 246341, 83824, 226962, 115962),
    (68603, 18235, 201699, 6558, 217064, 74053, 140307, 29320),
    (212222, 174163, 63891, 131714, 260991, 125525, 109871, 254552),
    (208133, 37817, 108871, 236086, 230829, 224735, 197202, 126789),
    (36220, 183667, 173531, 231574, 63007, 23270, 242256, 172824),
    (226174, 181177, 45094, 10219, 172720, 14537, 122494, 27364),
    (19288, 1130, 162371, 12239, 106820, 190833, 228451, 33845),
    (420, 256427, 250298, 234965, 137965, 33886, 192615, 137263),
    (30426, 206099, 1480, 169907, 122972, 5299, 178194, 116853),
    (38366, 252943, 119579, 233642, 99176, 152381, 1818, 246484),
    (49412, 124354, 252000, 221213, 103625, 2726, 153653, 148581),
    (82319, 1626, 107383, 158105, 81846, 13120, 1198, 193305),
    (44406, 239081, 240884, 84662, 7763, 52627, 182256, 187716),
    (185632, 105456, 212756, 173585, 81328, 74972, 128159, 45046),
    (104599, 7215, 61087, 26573, 59314, 48591, 945, 28553),
    (127710, 94893, 75476, 221733, 184125, 96685, 172243, 242612),
    (42647, 29769, 148111, 39823, 193859, 57502, 144317, 214559),
    (780, 145567, 79710, 226978, 2835, 160638, 8378, 24523),
    (161231, 246284, 44873, 150516, 114149, 68239, 117811, 141424),
    (31461, 110744, 232951, 16033, 179041, 106854, 47200, 63782),
    (255322, 241469, 248608, 95048, 170033, 253394, 261582, 181885),
    (63034, 5, 212309, 79222, 1841, 237107, 261430, 22474),
    (203738, 21095, 211942, 6233, 26825, 175918, 126433, 89713),
    (57893, 173681, 13566, 126980, 140303, 73406, 105028, 86705),
    (15800, 76765, 217596, 184873, 201602, 112166, 76158, 112065),
    (110522, 160113, 18684, 10469, 166599, 145226, 99589, 158310),
    (214726, 131223, 109288, 126812, 105792, 167086, 256918, 18441),
    (164736, 182565, 35066, 89660, 98586, 130539, 202194, 16684),
    (24903, 25959, 122313, 26525, 105627, 87218, 23062, 109362),
    (67552, 140412, 247510, 126439, 184322, 171107, 87397, 165128),
    (211326, 162921, 221946, 131793, 156106, 253917, 2345, 133918),
    (219591, 25610, 154884, 239521, 173390, 39973, 114213, 162088),
    (69694, 51180, 74827, 176121, 132947, 148345, 15083, 196459),
    (229624, 100015, 196100, 105569, 78527, 72176, 225549, 208691),
    (158498, 42753, 240006, 246065, 213196, 49877, 129372, 244273),
    (51001, 229538, 39704, 237637, 58774, 83576, 211231, 135814),
    (173630, 162748, 219633, 240928, 8298, 5311, 113776, 113251),
    (64061, 16436, 138070, 47525, 57016, 229742, 159929, 228539),
    (73108, 34503, 7538, 165920, 68681, 114191, 193009, 48042),
    (2842, 97501, 29489, 248778, 176907, 223147, 54452, 11731),
    (224345, 79068, 183290, 239324, 14912, 169078, 122283, 32914),
    (95340, 11646, 45163, 48387, 78062, 60978, 227735, 162106),
    (258986, 131616, 85766, 51383, 132449, 213013, 150516, 231609),
    (65332, 246689, 206208, 181886, 235636, 139183, 132468, 6602),
    (6778, 179487, 58159, 114248, 26277, 180706, 54969, 240497),
    (15413, 19595, 73952, 219244, 68813, 152629, 243501, 175077),
    (208668, 251169, 186627, 98857, 78225, 13125, 12392, 28954),
    (81754, 93281, 49839, 112579, 166016, 88571, 91558, 20863),
    (108264, 245898, 72992, 168504, 68263, 195879, 27596, 23576),
    (44918, 166098, 212537, 239555, 231283, 94408, 203172, 18701),
    (113563, 111669, 16481, 161974, 22111, 116384, 31096, 252828),
)


# ----------------------------------------------------------------------------
# device kernel builder


def build_kernel(n_img=IMG_PER_CORE, iters=ITERS, groups=GROUPS):
    nc = bacc.Bacc("TRN2", target_bir_lowering=False)

    XPW = 2 * CJ if SEG_FP8 else CJ    # xpix cols per group on device
    xpix_dt = F8 if SEG_FP8 else F16

    x5_d = nc.dram_tensor("x5", [n_img, NB * J, groups * P], F16,
                          kind="ExternalInput")
    xpix_d = nc.dram_tensor("xpix", [n_img, P, groups * XPW], xpix_dt,
                            kind="ExternalInput")
    SEGR_ = CJ if SEG_FP8 else 2 * CJ
    c0_d = nc.dram_tensor("cent0", [n_img, K, D], F32, kind="ExternalInput")
    diagk_d = nc.dram_tensor("diagk", [NB * J, FD], F32, kind="ExternalInput")
    csel_d = nc.dram_tensor("csel", [SEGR_, 4], F32, kind="ExternalInput")
    bca_d = nc.dram_tensor("bca", [5, NB * J], F32, kind="ExternalInput")
    bcb_d = nc.dram_tensor("bcb", [4, NB * J], F32, kind="ExternalInput")
    ident_d = nc.dram_tensor("ident", [K, K], F32, kind="ExternalInput")
    out_d = nc.dram_tensor("cent_out", [n_img, K, D], F32, kind="ExternalOutput")

    with tile.TileContext(nc) as tc:
        with (
            tc.tile_pool(name="singles", bufs=1) as singles,
            tc.tile_pool(name="s16p", bufs=3) as s16pool,
            tc.tile_pool(name="maskp", bufs=3) as maskpool,
            tc.tile_pool(name="mvp", bufs=1) as mvpool,
            tc.tile_pool(name="bigpsum", bufs=2, space="PSUM") as bigpsum,
            tc.tile_pool(name="segpsum", bufs=1, space="PSUM") as segpsum,
            tc.tile_pool(name="smallpsum", bufs=1, space="PSUM") as smallps,
        ):
            # --- constants ---
            diagk = singles.tile([NB * J, FD], F32, tag="diagk")
            nc.sync.dma_start(out=diagk[:], in_=diagk_d[:])
            diagk16 = singles.tile([NB * J, FD], F16, tag="diagk16")
            nc.scalar.copy(diagk16[:], diagk[:])
            csel = singles.tile([SEGR_, 4], F32, tag="csel")
            nc.sync.dma_start(out=csel[:], in_=csel_d[:])
            bca = singles.tile([5, NB * J], F32, tag="bca")
            nc.sync.dma_start(out=bca[:], in_=bca_d[:])
            bcb = singles.tile([4, NB * J], F32, tag="bcb")
            nc.sync.dma_start(out=bcb[:], in_=bcb_d[:])
            ident = singles.tile([K, K], F32, tag="ident")
            nc.sync.dma_start(out=ident[:], in_=ident_d[:])

            # --- persistent state ---
            # x5 lives in a 3-slot ring; xpix in a 4-slot ring. Two images
            # (a pair) are interleaved inside the trip loop so each image's
            # serial fold/update/weight chain hides under the other's batch
            # loop; the next pair prefetches during the current one.
            NX5 = 3
            x5t = [singles.tile([NB * J, groups * P], F16, tag=f"x5_{i}",
                                name=f"x5_{i}")
                   for i in range(NX5)]
            NXP = 3
            xpixt = [singles.tile([P, groups * XPW], xpix_dt, tag=f"xp_{i}",
                                  name=f"xp_{i}")
                     for i in range(NXP)]
            # per-pair-slot state (index 0 = image A, 1 = image B)
            cent = [singles.tile([K, D], F32, tag=f"cent{i}", name=f"cent{i}")
                    for i in range(2)]
            w8 = [singles.tile([K, 5], F32, tag=f"w8_{i}", name=f"w8_{i}")
                  for i in range(2)]
            for t in w8:
                nc.vector.memset(t[:, 4:5], 1.0)  # |x|^2 band weight
            csq = [singles.tile([K, D], F32, tag=f"csq{i}", name=f"csq{i}")
                   for i in range(2)]
            wt5 = [singles.tile([5, K], F32, tag=f"wt5_{i}", name=f"wt5_{i}")
                   for i in range(2)]
            whi16 = [singles.tile([4, K], F16, tag=f"whi{i}", name=f"whi{i}")
                     for i in range(2)]
            wlo = [singles.tile([4, K], F32, tag=f"wlo{i}", name=f"wlo{i}")
                   for i in range(2)]
            wrep = [singles.tile([NB * J, K], F16, tag=f"wrp{i}", name=f"wrp{i}")
                    for i in range(2)]
            wdiag16 = [singles.tile([NB * J, FD], F16, tag=f"wd{i}",
                                    name=f"wd{i}")
                       for i in range(2)]
            SEGR = CJ if SEG_FP8 else 2 * CJ   # fold row count
            prod = [singles.tile([SEGR, FD], F32, tag=f"prod{i}",
                                 name=f"prod{i}")
                    for i in range(2)]
            ext = [singles.tile([SEGR, K], F32, tag=f"ext{i}", name=f"ext{i}")
                   for i in range(2)]
            cntc = [singles.tile([K, 1], F32, tag=f"cnt{i}", name=f"cnt{i}")
                    for i in range(2)]
            recip = [singles.tile([K, 1], F32, tag=f"rcp{i}", name=f"rcp{i}")
                     for i in range(2)]
            pos = [singles.tile([K, 1], F32, tag=f"pos{i}", name=f"pos{i}")
                   for i in range(2)]
            cmean = [singles.tile([K, D], F32, tag=f"cm{i}", name=f"cm{i}")
                     for i in range(2)]
            cdel = [singles.tile([K, D], F32, tag=f"cd{i}", name=f"cd{i}")
                    for i in range(2)]

            NQ = 4

            def dma_image(img):
                x5b, xpb = x5t[img % NX5], xpixt[img % NXP]
                w = groups * P // NQ
                for q in range(NQ):
                    nc.sync.dma_start(
                        out=x5b[:, q * w:(q + 1) * w],
                        in_=x5_d[img][:, q * w:(q + 1) * w])
                w2 = groups * XPW // 2
                for q in range(2):
                    nc.sync.dma_start(
                        out=xpb[:, q * w2:(q + 1) * w2],
                        in_=xpix_d[img][:, q * w2:(q + 1) * w2])

            # persistent PSUM accumulators for the two in-flight images.
            # fp8 DoubleRow dst must sit at partition base 0, so the two
            # group-parity accumulation chains live side by side in the free
            # dim ([64, 2*FD]) instead of stacked on partitions.
            if SEG_FP8:
                segt = [segpsum.tile([CJ, 2 * FD], F32, tag=f"seg{i}",
                                     name=f"seg{i}")
                        for i in range(2)]
                segsum = [singles.tile([CJ, FD], F32, tag=f"ss{i}",
                                       name=f"ss{i}")
                          for i in range(2)]
            else:
                segt = [segpsum.tile([2 * CJ, FD], F32, tag=f"seg{i}",
                                     name=f"seg{i}")
                        for i in range(2)]

            def weights_part(sl):
                # ---- weights from centroids ----
                # w8 = [-2c | sum(c^2) | 1]; built on ACT to spare the DVE
                nc.scalar.mul(w8[sl][:, 0:D], cent[sl][:], -2.0)
                nc.scalar.activation(
                    csq[sl][:], cent[sl][:],
                    mybir.ActivationFunctionType.Square,
                    accum_out=w8[sl][:, 3:4])
                wtP = smallps.tile([5, K], F32, tag=f"small{sl}")
                nc.tensor.transpose(wtP[:], w8[sl][:], ident[:])
                nc.scalar.copy(wt5[sl][:], wtP[:])
                # lo-correction rows: wlo = wt - fp16(wt) for the -2c rows
                nc.scalar.copy(whi16[sl][:], wt5[sl][0:4, :])
                nc.vector.tensor_sub(wlo[sl][:], wt5[sl][0:4, :], whi16[sl][:])
                # wrep[(b,j), k] = per-band weight: bands 0-2 <- -2c,
                # 3-5 <- lo(-2c), 6 <- |c|^2, 7 <- 1 (two accumulated MMs
                # with constant selector matrices; no partition shifts)
                wrepP = smallps.tile([NB * J, K], F32, tag=f"small{sl}")
                nc.tensor.matmul(wrepP[:], bca[:], wt5[sl][:],
                                 start=True, stop=False)
                nc.tensor.matmul(wrepP[:], bcb[:], wlo[sl][:],
                                 start=False, stop=True)
                nc.scalar.copy(wrep[sl][:], wrepP[:])
                # wdiag16[(b,j), (k,j')] = wrep[(b,j), k] * 1[j==j']
                # (all-fp16 operands keep the DVE in 2x packed mode; the
                # fp16 rounding of wrep matches wdiag16's own rounding)
                wrep_b = bass.AP(
                    tensor=wrep[sl][:].tensor, offset=wrep[sl][:].offset,
                    ap=[wrep[sl][:].ap[0], [1, K], [0, J]])
                nc.vector.tensor_tensor(
                    wdiag16[sl][:].rearrange("p (k j) -> p k j", j=J),
                    diagk16[:].rearrange("p (k j) -> p k j", j=J),
                    wrep_b, mybir.AluOpType.mult)

            def batches_part(sl, x5b, xpb, carry=None, hook=None):
                # ---- main loop over super-batches. Seg matmuls trail their
                # mask by one super-batch, flushed in 8-MM chunks between
                # score bursts so the PE interleaves scores and seg work and
                # copies are never head-of-line blocked. The final super-
                # batch's chunks are RETURNED and flush inside the OTHER
                # image's batch loop (cross-image carry); `hook` emits that
                # image's fold/weights chain once the carry has drained. ----
                seg = segt[sl]
                flushq = list(carry) if carry else []

                if SEG_FP8:
                    def seg_chunk(mk, q, c):
                        def go():
                            mk8 = mk[:].bitcast(F8).rearrange(
                                "p (n two) -> p two n", two=2)
                            for t in range(c * 8, c * 8 + 8):
                                g = q * SDB + t
                                hf = g & 1
                                xp8 = xpb[:, g * XPW:(g + 1) * XPW].rearrange(
                                    "p (two c) -> p two c", two=2)
                                nc.tensor.matmul(
                                    seg[:, hf * FD:(hf + 1) * FD],
                                    xp8,
                                    mk8[:, :, t * FD:(t + 1) * FD],
                                    perf_mode=mybir.MatmulPerfMode.DoubleRow,
                                    start=(g == hf),
                                    stop=(g == groups - 2 + hf),
                                    skip_group_check=True)
                        return go
                else:
                    def seg_chunk(mk, q, c):
                        def go():
                            for t in range(c * 8, c * 8 + 8):
                                g = q * SDB + t
                                hf = g & 1
                                nc.tensor.matmul(
                                    seg[hf * CJ:(hf + 1) * CJ, :],
                                    xpb[:, g * CJ:(g + 1) * CJ],
                                    mk[:, t * FD:(t + 1) * FD],
                                    start=(g == hf),
                                    stop=(g == groups - 2 + hf),
                                    skip_group_check=True,
                                    tile_position=(0, hf * CJ))
                        return go

                for q in range(NSUP):
                    s16 = s16pool.tile([P, SDB * FD], F16, tag="s16")
                    if q == 1 and hook is not None:
                        hook()
                    for h in range(4):
                        sp = bigpsum.tile([P, GBATCH * FD], F32, tag="big")
                        for t in range(GBATCH):
                            g = (q * 4 + h) * GBATCH + t
                            nc.tensor.matmul(
                                sp[:, t * FD:(t + 1) * FD],
                                x5b[:, g * P:(g + 1) * P],
                                wdiag16[sl][:], start=True, stop=True)
                        if flushq:
                            flushq.pop(0)()
                        # ACT evacuates scores to fp16 SBUF (k-outer layout)
                        nc.scalar.copy(
                            s16[:, h * GBATCH * FD:(h + 1) * GBATCH * FD],
                            sp[:])
                    s4 = s16[:].rearrange("p (gb k j) -> p gb k j", k=K, j=J)
                    # 3-round pairwise min over k (all 2x packed-16 mode)
                    mv1 = mvpool.tile([P, SDB * 4 * J], F16, tag="mv1")
                    m1 = mv1[:].rearrange("p (gb k j) -> p gb k j", k=4, j=J)
                    nc.vector.tensor_tensor(
                        m1, s4[:, :, 0:4, :], s4[:, :, 4:8, :],
                        mybir.AluOpType.min)
                    mv2 = mvpool.tile([P, SDB * 2 * J], F16, tag="mv2")
                    m2 = mv2[:].rearrange("p (gb k j) -> p gb k j", k=2, j=J)
                    nc.vector.tensor_tensor(
                        m2, m1[:, :, 0:2, :], m1[:, :, 2:4, :],
                        mybir.AluOpType.min)
                    mv3 = mvpool.tile([P, SDB * J], F16, tag="mv3")
                    m3 = mv3[:].rearrange("p (gb j) -> p gb j", j=J)
                    nc.vector.tensor_tensor(
                        m3.unsqueeze(2), m2[:, :, 0:1, :], m2[:, :, 1:2, :],
                        mybir.AluOpType.min)
                    # mask = (s16 <= min) broadcast over k
                    mv_b = bass.AP(
                        tensor=mv3[:].tensor, offset=mv3[:].offset,
                        ap=[mv3[:].ap[0], [J, SDB], [0, K], [1, J]])
                    mk = maskpool.tile([P, SDB * FD], F16, tag="mk")
                    nc.vector.tensor_tensor(
                        mk[:].rearrange("p (gb k j) -> p gb k j", k=K, j=J),
                        s4, mv_b, mybir.AluOpType.is_le)
                    flushq.extend(seg_chunk(mk, q, c) for c in range(4))
                return flushq

            def fold_update_part(sl):
                # ---- fold seg -> S[k, (r,g,b,count)] ----
                if SEG_FP8:
                    # sum the two parity chains, then one 64-row fold
                    # (only one PSUM operand allowed per instruction)
                    nc.scalar.copy(segsum[sl][:], segt[sl][:, 0:FD])
                    nc.vector.tensor_add(
                        segsum[sl][:], segsum[sl][:], segt[sl][:, FD:])
                    nc.vector.tensor_tensor(
                        prod[sl][:], segsum[sl][:], diagk[0:CJ, :],
                        mybir.AluOpType.mult)
                else:
                    nc.vector.tensor_tensor(
                        prod[sl][:], segt[sl][:], diagk[:],
                        mybir.AluOpType.mult)
                nc.vector.tensor_reduce(
                    ext[sl][:],
                    prod[sl][:].rearrange("p (k j) -> p k j", j=J),
                    axis=mybir.AxisListType.X,
                    op=mybir.AluOpType.add)
                S = smallps.tile([K, 4], F32, tag=f"small{sl}")
                nc.tensor.matmul(S[:], ext[sl][:], csel[:],
                                 start=True, stop=True)

                # ---- centroid update ----
                # counts come back scaled by 1.5 in fp8 mode; threshold 0.5
                # keeps n=0 -> mean 0 (masked) and n>=1 exact in both modes.
                nc.vector.tensor_scalar_max(cntc[sl][:], S[:, 3:4], 0.5)
                nc.vector.reciprocal(recip[sl][:], cntc[sl][:])
                # per-partition scale rides the ACT ops (spares the DVE)
                nc.scalar.mul(cmean[sl][:], S[:, 0:D], recip[sl][:])
                nc.vector.tensor_scalar(
                    pos[sl][:], S[:, 3:4], 0.5, None,
                    op0=mybir.AluOpType.is_ge)
                nc.vector.tensor_sub(cdel[sl][:], cmean[sl][:], cent[sl][:])
                nc.scalar.mul(cdel[sl][:], cdel[sl][:], pos[sl][:])
                nc.vector.tensor_add(cent[sl][:], cent[sl][:], cdel[sl][:])

            nc.sync.dma_start(out=cent[0][:], in_=c0_d[0])
            nc.sync.dma_start(out=cent[1][:], in_=c0_d[1])
            dma_image(0)
            dma_image(1)
            weights_part(0)              # pair 0's A weights

            for pair in range(n_img // 2):
                a, b = 2 * pair, 2 * pair + 1
                # prefetch next pair's A image; its x5/xpix slots are
                # unused by this pair. B's slot aliases image a's, so its
                # prefetch is issued after the trip loop below.
                if a + 2 < n_img:
                    dma_image(a + 2)

                xa, pa = x5t[a % NX5], xpixt[a % NXP]
                xb, pb = x5t[b % NX5], xpixt[b % NXP]

                # Fully symmetric software pipeline: each image's
                # fold/update/weights chain is emitted from a hook INSIDE
                # the other image's batch loop (after the carried seg
                # chunks drain), and each image's final super-batch of seg
                # matmuls flushes interleaved with the other image's score
                # bursts. On the last trip, A's result ships early and the
                # NEXT pair's A-centroids + weight chain are built under
                # B's final batch loop.
                def hook_b0():
                    weights_part(1)

                def hook_a():
                    fold_update_part(1)
                    weights_part(1)

                def hook_b():
                    fold_update_part(0)
                    weights_part(0)

                def hook_b_last():
                    fold_update_part(0)
                    nc.sync.dma_start(out=out_d[a], in_=cent[0][:])
                    if a + 2 < n_img:
                        nc.sync.dma_start(out=cent[0][:], in_=c0_d[a + 2])
                        weights_part(0)   # next pair's A weights

                carry = batches_part(0, xa, pa, hook=hook_b0)
                carry = batches_part(1, xb, pb, carry=carry, hook=hook_b)
                for t in range(1, iters):
                    carry = batches_part(0, xa, pa, carry=carry, hook=hook_a)
                    carry = batches_part(
                        1, xb, pb, carry=carry,
                        hook=hook_b if t < iters - 1 else hook_b_last)
                for chunk in carry:
                    chunk()
                fold_update_part(1)

                nc.sync.dma_start(out=out_d[b], in_=cent[1][:])
                if b + 2 < n_img:
                    nc.sync.dma_start(out=cent[1][:], in_=c0_d[b + 2])
                    dma_image(b + 2)

    nc.finalize()
    return nc


# ----------------------------------------------------------------------------
# host-side layouts


def host_layouts(pixels):
    """pixels [B, N, 3] f32 -> (x5 [B, 128, 16384] f16, xpix fp8/fp16).

    x5[(b,j), (g,p)]: bands 0-2 / 3-5 = x_rgb fp16 (hi/lo share data),
    band 6 = 1.0, band 7 = |x|^2 fp16, for pixel g*J*P + j*P + p.
    xpix fp8 mode: [p, (g, s, c, j)], s=0 zeros (pairs with the always-zero
    low mask byte), s=1 = fp8 pixel values; c in {r,g,b,1}.
    """
    b = pixels.shape[0]
    g = GROUPS
    v = pixels.reshape(b, g, J, P, D)
    rgb = np.ascontiguousarray(
        v.transpose(0, 4, 2, 1, 3).reshape(b, D * J, g * P)).astype(np.float16)
    xsq = (pixels.astype(np.float32) ** 2).sum(-1).astype(np.float16)
    xsqr = np.ascontiguousarray(
        xsq.reshape(b, g, J, P).transpose(0, 2, 1, 3).reshape(b, J, g * P))
    x5 = np.empty((b, NB * J, g * P), np.float16)
    x5[:, 0:48] = rgb
    x5[:, 48:96] = rgb
    x5[:, 96:112] = np.float16(1.0)
    x5[:, 112:128] = xsqr
    if SEG_FP8:
        xp = np.zeros((b, P, g, 2, 4, J), ml_dtypes.float8_e4m3)
        xp[..., 1, 0:3, :] = v.transpose(0, 3, 1, 4, 2).astype(
            ml_dtypes.float8_e4m3)  # b p g c j
        xp[..., 1, 3, :] = ml_dtypes.float8_e4m3(1.0)
        xpix = np.ascontiguousarray(xp.reshape(b, P, g * 2 * CJ))
    else:
        xp = np.empty((b, P, g, 4, J), np.float16)
        xp[..., 0:3, :] = v.transpose(0, 3, 1, 4, 2).astype(np.float16)
        xp[..., 3, :] = np.float16(1.0)
        xpix = np.ascontiguousarray(xp.reshape(b, P, g * CJ))
    return x5, xpix


def host_constants():
    diagk = np.zeros((NB * J, FD), np.float32)
    for bnd in range(NB):
        for j in range(J):
            for k in range(K):
                diagk[bnd * J + j, k * J + j] = 1.0
    if SEG_FP8:
        csel = np.zeros((CJ, 4), np.float32)
        for c in range(4):
            for j in range(J):
                csel[c * J + j, c] = 1.0
    else:
        csel = np.zeros((2 * CJ, 4), np.float32)
        for h in range(2):
            for c in range(4):
                for j in range(J):
                    csel[h * CJ + c * J + j, c] = 1.0
    # bca: wt5 rows (-2cx,-2cy,-2cz, cc, 1) -> bands (0,1,2, 6, 7)
    # bcb: wlo rows (lox,loy,loz, junk) -> bands (3,4,5, -)
    bca = np.zeros((5, NB * J), np.float32)
    bcb = np.zeros((4, NB * J), np.float32)
    for j in range(J):
        for r, bnd in enumerate((0, 1, 2, 6, 7)):
            bca[r, bnd * J + j] = 1.0
        for r, bnd in enumerate((3, 4, 5)):
            bcb[r, bnd * J + j] = 1.0
    ident = np.eye(K, dtype=np.float32)
    return diagk, csel, bca, bcb, ident


_NC_CACHE = {}
TRACE = False
LAST_RESULTS = None


def _get_nc(n_img, iters, groups):
    key = (n_img, iters, groups)
    if key not in _NC_CACHE:
        _NC_CACHE[key] = build_kernel(n_img, iters, groups)
    return _NC_CACHE[key]


def kernel(inputs: np.ndarray) -> np.ndarray:
    x = np.ascontiguousarray(np.asarray(inputs, dtype=np.float32))
    assert x.shape == (B, H, W, D), x.shape
    pixels = x.reshape(B, N, D)

    perm8 = np.array(PERM8, dtype=np.int64)
    cent0 = np.take_along_axis(
        pixels, perm8[:, :, None].repeat(D, axis=2), axis=1
    ).astype(np.float32)

    x5, xpix = host_layouts(pixels)
    diagk, csel, bca, bcb, ident = host_constants()
    nc = _get_nc(IMG_PER_CORE, ITERS, GROUPS)

    in_maps = []
    for c in range(NCORES):
        sl = slice(c * IMG_PER_CORE, (c + 1) * IMG_PER_CORE)
        in_maps.append({
            "x5": np.ascontiguousarray(x5[sl]),
            "xpix": np.ascontiguousarray(xpix[sl]),
            "cent0": np.ascontiguousarray(cent0[sl]),
            "diagk": diagk,
            "csel": csel,
            "bca": bca,
            "bcb": bcb,
            "ident": ident,
        })

    global LAST_RESULTS
    try:
        res = run_bass_kernel_spmd(nc, in_maps, core_ids=list(range(NCORES)),
                                   trace=TRACE)
    except Exception:
        if not TRACE:
            raise
        res = run_bass_kernel_spmd(nc, in_maps, core_ids=list(range(NCORES)))
    LAST_RESULTS = res
    outs = [r["cent_out"].reshape(IMG_PER_CORE, K * D) for r in res.results]
    return np.concatenate(outs, axis=0).astype(np.float32)


if __name__ == "__main__":
    rs = np.random.RandomState(0)
    x = rs.random_sample((B, H, W, D)).astype(np.float32)
    out = kernel(inputs=x)
    print("out shape", out.shape, out.dtype)
    print(out[0])


# revision 22
# speedup vs baseline: 1.0008x; 1.0008x over previous
"""Trainium2 Bass kernel for nn_ColorExtractor (per-image k-means, K=8, 10 iters).

Contract: kernel(**inputs) takes FULL inputs ([64, 512, 512, 3] f32), returns
FULL output ([64, 24] f32), batch sharded over 8 NeuronCores (8 images/core).

v3 design (vs v2):
  seg     segment-sum matmuls run in fp8 DoubleRow mode at 2x PE rate: the
          fp16 0/1 mask buffer is bitcast to fp8e4 pairs (lo byte always
          0x00 -> 0.0, hi byte 0x3C -> 1.5) and fed as the two DoubleRow
          contraction subtiles; xpix is stored fp8 with a zeroed partner
          subtile. Sums come out scaled by exactly 1.5, which cancels in
          means; the count threshold drops to 0.5 (n=0 -> masked anyway).
  argmin  DVE work batched at 32-group granularity (half the instruction
          dispatch of v2); min tree + is_le unchanged otherwise.
  weights w8 = [-2c | sum c^2] built on ACT (mul + Square-with-accum),
          freeing DVE cycles.

Initial centroids replicate jax.random.permutation(key, N)[:8] via the
precomputed PERM8 table (numpy threefry port, verified bit-exact).
"""

import numpy as np

try:                      # only needed for the (disabled) fp8 seg path
    import ml_dtypes
except ImportError:       # pragma: no cover
    ml_dtypes = None

import concourse.bacc as bacc
import concourse.bass as bass
import concourse.tile as tile
from concourse import mybir
from concourse.bass_utils import run_bass_kernel_spmd

# ----------------------------------------------------------------------------
# problem constants (hardcoded per contract)
B = 64            # total images
NCORES = 8
IMG_PER_CORE = B // NCORES
H = W = 512
N = H * W         # pixels per image: 262144
K = 8             # clusters
ITERS = 10
D = 3

# device tiling
P = 128           # pixels per chunk
J = 16            # chunks per matmul group (block-diag j packing)
NB = 8            # contraction bands: r,g,b, r,g,b(lo), ones, |x|^2
GROUPS = N // (J * P)    # 128 groups per image
GBATCH = 8        # groups per PSUM batch
SDB = 4 * GBATCH  # groups per DVE super-batch (4 PSUM fills)
NSUP = GROUPS // SDB     # 4
FD = K * J        # 128: free dim of scores/seg matmuls, cols = (k, j)
CJ = 4 * J        # 64: xpix cols per group, (c in {r,g,b,1}, j)

F32 = mybir.dt.float32
F16 = mybir.dt.float16
F8 = mybir.dt.float8e4

# fp8 DoubleRow segment sums (mask byte-alias trick): measured rel err
# 1.98e-2 vs the 2e-2 gate (fp8 xpix quantization excites trajectory chaos
# in k-means) and the DR matmuls serialize LDWEIGHTS. Keep the fp16 path.
SEG_FP8 = False

# ----------------------------------------------------------------------------
# numpy threefry port (verified bit-exact vs jax 0.8 threefry2x32 impl)
_U32 = np.uint32


def _rotl(x, d):
    d = _U32(d)
    return (x << d) | (x >> _U32(32 - d))


def _threefry2x32(k1, k2, x1, x2):
    with np.errstate(over="ignore"):
        ks0, ks1 = _U32(k1), _U32(k2)
        ks2 = _U32(ks0 ^ ks1 ^ _U32(0x1BD11BDA))
        x = [(x1 + ks0).astype(_U32), (x2 + ks1).astype(_U32)]

        def rounds(rots, ka, kb, inc):
            for r in rots:
                x[0] = (x[0] + x[1]).astype(_U32)
                x[1] = _rotl(x[1], r)
                x[1] = x[0] ^ x[1]
            x[0] = (x[0] + ka).astype(_U32)
            x[1] = (x[1] + kb + _U32(inc)).astype(_U32)

        rounds((13, 15, 26, 6), ks1, ks2, 1)
        rounds((17, 29, 16, 24), ks2, ks0, 2)
        rounds((13, 15, 26, 6), ks0, ks1, 3)
        rounds((17, 29, 16, 24), ks1, ks2, 4)
        rounds((13, 15, 26, 6), ks2, ks0, 5)
    return x[0], x[1]


def _tf_split(key, num):
    i = np.arange(num, dtype=np.uint64)
    b1, b2 = _threefry2x32(key[0], key[1],
                           (i >> np.uint64(32)).astype(_U32), i.astype(_U32))
    return np.stack([b1, b2], axis=1)


def _tf_bits(key, n):
    i = np.arange(n, dtype=np.uint64)
    b1, b2 = _threefry2x32(key[0], key[1],
                           (i >> np.uint64(32)).astype(_U32), i.astype(_U32))
    return b1 ^ b2


def jax_permutation_indices(seed, batch, n):
    """perm[b] = jax.random.permutation(split(key(seed), batch)[b], n)."""
    keys = _tf_split(np.array([0, seed], _U32), batch)
    num_rounds = int(np.ceil(3 * np.log(max(1, n)) / np.log(2**32 - 1)))
    perms = []
    for b in range(batch):
        x = np.arange(n)
        k = keys[b]
        for _ in range(num_rounds):
            ks = _tf_split(k, 2)
            k = ks[0]
            sort_keys = _tf_bits(ks[1], n)
            x = x[np.argsort(sort_keys, kind="stable")]
        perms.append(x[:K])
    return np.stack(perms)  # [batch, K]


# Precomputed jax.random.permutation(split(key(42), 64)[b], N)[:8] indices
# (input-independent; verified against the threefry port above).
PERM8 = (
    (121373, 128858, 64733, 199519, 198377, 234239, 198325, 209106),
    (73520, 236184, 209288, 97370, 64322, 228694, 126128, 72161),
    (143944, 27877, 97040, 2149, 10994, 109181, 179954, 54887),
    (147613, 8773, 54262, 44295, 29289, 11407, 31612, 133442),
    (206432, 166428, 5023, 212109, 16365, 21194, 249053, 195143),
    (13257, 110295, 84080, 119151, 246640, 69532, 130091, 105945),
    (14760, 174397, 198857, 826, 140745, 258776, 214608, 163989),
    (184593, 240934, 160738, 23779, 43199, 47433, 94941, 50416),
    (4386, 21260, 129661, 125128, 50701, 200388, 254109, 44816),
    (203980, 230711, 102351, 31296, 161690, 63692, 194032, 60281),
    (170168, 75997, 12072, 137876, 34146, 48636, 181597, 67859),
    (218987, 48148, 224774, 27163, 85280, 163529, 107708, 238871),
    (152153, 120028, 50368, 168498, 254864, 185234, 259971, 5221),
    (126051, 57270, 7614, 194865, 246341, 83824, 226962, 115962),
    (68603, 18235, 201699, 6558, 217064, 74053, 140307, 29320),
    (212222, 174163, 63891, 131714, 260991, 125525, 109871, 254552),
    (208133, 37817, 108871, 236086, 230829, 224735, 197202, 126789),
    (36220, 183667, 173531, 231574, 63007, 23270, 242256, 172824),
    (226174, 181177, 45094, 10219, 172720, 14537, 122494, 27364),
    (19288, 1130, 162371, 12239, 106820, 190833, 228451, 33845),
    (420, 256427, 250298, 234965, 137965, 33886, 192615, 137263),
    (30426, 206099, 1480, 169907, 122972, 5299, 178194, 116853),
    (38366, 252943, 119579, 233642, 99176, 152381, 1818, 246484),
    (49412, 124354, 252000, 221213, 103625, 2726, 153653, 148581),
    (82319, 1626, 107383, 158105, 81846, 13120, 1198, 193305),
    (44406, 239081, 240884, 84662, 7763, 52627, 182256, 187716),
    (185632, 105456, 212756, 173585, 81328, 74972, 128159, 45046),
    (104599, 7215, 61087, 26573, 59314, 48591, 945, 28553),
    (127710, 94893, 75476, 221733, 184125, 96685, 172243, 242612),
    (42647, 29769, 148111, 39823, 193859, 57502, 144317, 214559),
    (780, 145567, 79710, 226978, 2835, 160638, 8378, 24523),
    (161231, 246284, 44873, 150516, 114149, 68239, 117811, 141424),
    (31461, 110744, 232951, 16033, 179041, 106854, 47200, 63782),
    (255322, 241469, 248608, 95048, 170033, 253394, 261582, 181885),
    (63034, 5, 212309, 79222, 1841, 237107, 261430, 22474),
    (203738, 21095, 211942, 6233, 26825, 175918, 126433, 89713),
    (57893, 173681, 13566, 126980, 140303, 73406, 105028, 86705),
    (15800, 76765, 217596, 184873, 201602, 112166, 76158, 112065),
    (110522, 160113, 18684, 10469, 166599, 145226, 99589, 158310),
    (214726, 131223, 109288, 126812, 105792, 167086, 256918, 18441),
    (164736, 182565, 35066, 89660, 98586, 130539, 202194, 16684),
    (24903, 25959, 122313, 26525, 105627, 87218, 23062, 109362),
    (67552, 140412, 247510, 126439, 184322, 171107, 87397, 165128),
    (211326, 162921, 221946, 131793, 156106, 253917, 2345, 133918),
    (219591, 25610, 154884, 239521, 173390, 39973, 114213, 162088),
    (69694, 51180, 74827, 176121, 132947, 148345, 15083, 196459),
    (229624, 100015, 196100, 105569, 78527, 72176, 225549, 208691),
    (158498, 42753, 240006, 246065, 213196, 49877, 129372, 244273),
    (51001, 229538, 39704, 237637, 58774, 83576, 211231, 135814),
    (173630, 162748, 219633, 240928, 8298, 5311, 113776, 113251),
    (64061, 16436, 138070, 47525, 57016, 229742, 159929, 228539),
    (73108, 34503, 7538, 165920, 68681, 114191, 193009, 48042),
    (2842, 97501, 29489, 248778, 176907, 223147, 54452, 11731),
    (224345, 79068, 183290, 239324, 14912, 169078, 122283, 32914),
    (95340, 11646, 45163, 48387, 78062, 60978, 227735, 162106),
    (258986, 131616, 85766, 51383, 132449, 213013, 150516, 231609),
    (65332, 246689, 206208, 181886, 235636, 139183, 132468, 6602),
    (6778, 179487, 58159, 114248, 26277, 180706, 54969, 240497),
    (15413, 19595, 73952, 219244, 68813, 152629, 243501, 175077),
    (208668, 251169, 186627, 98857, 78225, 13125, 12392, 28954),
    (81754, 93281, 49839, 112579, 166016, 88571, 91558, 20863),
    (108264, 245898, 72992, 168504, 68263, 195879, 27596, 23576),
    (44918, 166098, 212537, 239555, 231283, 94408, 203172, 18701),
    (113563, 111669, 16481, 161974, 22111, 116384, 31096, 252828),
)


# ----------------------------------------------------------------------------
# device kernel builder


def build_kernel(n_img=IMG_PER_CORE, iters=ITERS, groups=GROUPS):
    nc = bacc.Bacc("TRN2", target_bir_lowering=False)

    XPW = 2 * CJ if SEG_FP8 else CJ    # xpix cols per group on device
    xpix_dt = F8 if SEG_FP8 else F16

    x5_d = nc.dram_tensor("x5", [n_img, NB * J, groups * P], F16,
                          kind="ExternalInput")
    xpix_d = nc.dram_tensor("xpix", [n_img, P, groups * XPW], xpix_dt,
                            kind="ExternalInput")
    SEGR_ = CJ if SEG_FP8 else 2 * CJ
    c0_d = nc.dram_tensor("cent0", [n_img, K, D], F32, kind="ExternalInput")
    diagk_d = nc.dram_tensor("diagk", [NB * J, FD], F32, kind="ExternalInput")
    csel_d = nc.dram_tensor("csel", [SEGR_, 4], F32, kind="ExternalInput")
    bca_d = nc.dram_tensor("bca", [5, NB * J], F32, kind="ExternalInput")
    bcb_d = nc.dram_tensor("bcb", [4, NB * J], F32, kind="ExternalInput")
    ident_d = nc.dram_tensor("ident", [K, K], F32, kind="ExternalInput")
    out_d = nc.dram_tensor("cent_out", [n_img, K, D], F32, kind="ExternalOutput")

    with tile.TileContext(nc) as tc:
        with (
            tc.tile_pool(name="singles", bufs=1) as singles,
            tc.tile_pool(name="s16p", bufs=3) as s16pool,
            tc.tile_pool(name="maskp", bufs=3) as maskpool,
            tc.tile_pool(name="mvp", bufs=1) as mvpool,
            tc.tile_pool(name="bigpsum", bufs=2, space="PSUM") as bigpsum,
            tc.tile_pool(name="segpsum", bufs=1, space="PSUM") as segpsum,
            tc.tile_pool(name="smallpsum", bufs=1, space="PSUM") as smallps,
        ):
            # --- constants ---
            diagk = singles.tile([NB * J, FD], F32, tag="diagk")
            nc.sync.dma_start(out=diagk[:], in_=diagk_d[:])
            diagk16 = singles.tile([NB * J, FD], F16, tag="diagk16")
            nc.scalar.copy(diagk16[:], diagk[:])
            csel = singles.tile([SEGR_, 4], F32, tag="csel")
            nc.sync.dma_start(out=csel[:], in_=csel_d[:])
            bca = singles.tile([5, NB * J], F32, tag="bca")
            nc.sync.dma_start(out=bca[:], in_=bca_d[:])
            bcb = singles.tile([4, NB * J], F32, tag="bcb")
            nc.sync.dma_start(out=bcb[:], in_=bcb_d[:])
            ident = singles.tile([K, K], F32, tag="ident")
            nc.sync.dma_start(out=ident[:], in_=ident_d[:])

            # --- persistent state ---
            # x5 lives in a 3-slot ring; xpix in a 4-slot ring. Two images
            # (a pair) are interleaved inside the trip loop so each image's
            # serial fold/update/weight chain hides under the other's batch
            # loop; the next pair prefetches during the current one.
            NX5 = 3
            x5t = [singles.tile([NB * J, groups * P], F16, tag=f"x5_{i}",
                                name=f"x5_{i}")
                   for i in range(NX5)]
            NXP = 3
            xpixt = [singles.tile([P, groups * XPW], xpix_dt, tag=f"xp_{i}",
                                  name=f"xp_{i}")
                     for i in range(NXP)]
            # per-pair-slot state (index 0 = image A, 1 = image B)
            cent = [singles.tile([K, D], F32, tag=f"cent{i}", name=f"cent{i}")
                    for i in range(2)]
            w8 = [singles.tile([K, 5], F32, tag=f"w8_{i}", name=f"w8_{i}")
                  for i in range(2)]
            for t in w8:
                nc.vector.memset(t[:, 4:5], 1.0)  # |x|^2 band weight
            csq = [singles.tile([K, D], F32, tag=f"csq{i}", name=f"csq{i}")
                   for i in range(2)]
            wt5 = [singles.tile([5, K], F32, tag=f"wt5_{i}", name=f"wt5_{i}")
                   for i in range(2)]
            whi16 = [singles.tile([4, K], F16, tag=f"whi{i}", name=f"whi{i}")
                     for i in range(2)]
            wlo = [singles.tile([4, K], F32, tag=f"wlo{i}", name=f"wlo{i}")
                   for i in range(2)]
            wrep = [singles.tile([NB * J, K], F16, tag=f"wrp{i}", name=f"wrp{i}")
                    for i in range(2)]
            wdiag16 = [singles.tile([NB * J, FD], F16, tag=f"wd{i}",
                                    name=f"wd{i}")
                       for i in range(2)]
            SEGR = CJ if SEG_FP8 else 2 * CJ   # fold row count
            prod = [singles.tile([SEGR, FD], F32, tag=f"prod{i}",
                                 name=f"prod{i}")
                    for i in range(2)]
            ext = [singles.tile([SEGR, K], F32, tag=f"ext{i}", name=f"ext{i}")
                   for i in range(2)]
            cntc = [singles.tile([K, 1], F32, tag=f"cnt{i}", name=f"cnt{i}")
                    for i in range(2)]
            recip = [singles.tile([K, 1], F32, tag=f"rcp{i}", name=f"rcp{i}")
                     for i in range(2)]
            pos = [singles.tile([K, 1], F32, tag=f"pos{i}", name=f"pos{i}")
                   for i in range(2)]
            cmean = [singles.tile([K, D], F32, tag=f"cm{i}", name=f"cm{i}")
                     for i in range(2)]
            cdel = [singles.tile([K, D], F32, tag=f"cd{i}", name=f"cd{i}")
                    for i in range(2)]

            NQ = 4

            def dma_image(img):
                x5b, xpb = x5t[img % NX5], xpixt[img % NXP]
                w = groups * P // NQ
                for q in range(NQ):
                    nc.sync.dma_start(
                        out=x5b[:, q * w:(q + 1) * w],
                        in_=x5_d[img][:, q * w:(q + 1) * w])
                w2 = groups * XPW // 2
                for q in range(2):
                    nc.sync.dma_start(
                        out=xpb[:, q * w2:(q + 1) * w2],
                        in_=xpix_d[img][:, q * w2:(q + 1) * w2])

            # persistent PSUM accumulators for the two in-flight images.
            # fp8 DoubleRow dst must sit at partition base 0, so the two
            # group-parity accumulation chains live side by side in the free
            # dim ([64, 2*FD]) instead of stacked on partitions.
            if SEG_FP8:
                segt = [segpsum.tile([CJ, 2 * FD], F32, tag=f"seg{i}",
                                     name=f"seg{i}")
                        for i in range(2)]
                segsum = [singles.tile([CJ, FD], F32, tag=f"ss{i}",
                                       name=f"ss{i}")
                          for i in range(2)]
            else:
                segt = [segpsum.tile([2 * CJ, FD], F32, tag=f"seg{i}",
                                     name=f"seg{i}")
                        for i in range(2)]

            def weights_part(sl):
                # ---- weights from centroids ----
                # w8 = [-2c | sum(c^2) | 1]; built on ACT to spare the DVE
                nc.scalar.mul(w8[sl][:, 0:D], cent[sl][:], -2.0)
                nc.scalar.activation(
                    csq[sl][:], cent[sl][:],
                    mybir.ActivationFunctionType.Square,
                    accum_out=w8[sl][:, 3:4])
                wtP = smallps.tile([5, K], F32, tag=f"small{sl}")
                nc.tensor.transpose(wtP[:], w8[sl][:], ident[:])
                nc.scalar.copy(wt5[sl][:], wtP[:])
                # lo-correction rows: wlo = wt - fp16(wt) for the -2c rows
                nc.scalar.copy(whi16[sl][:], wt5[sl][0:4, :])
                nc.vector.tensor_sub(wlo[sl][:], wt5[sl][0:4, :], whi16[sl][:])
                # wrep[(b,j), k] = per-band weight: bands 0-2 <- -2c,
                # 3-5 <- lo(-2c), 6 <- |c|^2, 7 <- 1 (two accumulated MMs
                # with constant selector matrices; no partition shifts)
                wrepP = smallps.tile([NB * J, K], F32, tag=f"small{sl}")
                nc.tensor.matmul(wrepP[:], bca[:], wt5[sl][:],
                                 start=True, stop=False)
                nc.tensor.matmul(wrepP[:], bcb[:], wlo[sl][:],
                                 start=False, stop=True)
                nc.scalar.copy(wrep[sl][:], wrepP[:])
                # wdiag16[(b,j), (k,j')] = wrep[(b,j), k] * 1[j==j']
                # (all-fp16 operands keep the DVE in 2x packed mode; the
                # fp16 rounding of wrep matches wdiag16's own rounding)
                wrep_b = bass.AP(
                    tensor=wrep[sl][:].tensor, offset=wrep[sl][:].offset,
                    ap=[wrep[sl][:].ap[0], [1, K], [0, J]])
                nc.vector.tensor_tensor(
                    wdiag16[sl][:].rearrange("p (k j) -> p k j", j=J),
                    diagk16[:].rearrange("p (k j) -> p k j", j=J),
                    wrep_b, mybir.AluOpType.mult)

            def batches_part(sl, x5b, xpb, carry=None, hook=None):
                # ---- main loop over super-batches. Seg matmuls trail their
                # mask by one super-batch, flushed in 8-MM chunks between
                # score bursts so the PE interleaves scores and seg work and
                # copies are never head-of-line blocked. The final super-
                # batch's chunks are RETURNED and flush inside the OTHER
                # image's batch loop (cross-image carry); `hook` emits that
                # image's fold/weights chain once the carry has drained. ----
                seg = segt[sl]
                flushq = list(carry) if carry else []

                if SEG_FP8:
                    def seg_chunk(mk, q, c):
                        def go():
                            mk8 = mk[:].bitcast(F8).rearrange(
                                "p (n two) -> p two n", two=2)
                            for t in range(c * 8, c * 8 + 8):
                                g = q * SDB + t
                                hf = g & 1
                                xp8 = xpb[:, g * XPW:(g + 1) * XPW].rearrange(
                                    "p (two c) -> p two c", two=2)
                                nc.tensor.matmul(
                                    seg[:, hf * FD:(hf + 1) * FD],
                                    xp8,
                                    mk8[:, :, t * FD:(t + 1) * FD],
                                    perf_mode=mybir.MatmulPerfMode.DoubleRow,
                                    start=(g == hf),
                                    stop=(g == groups - 2 + hf),
                                    skip_group_check=True)
                        return go
                else:
                    def seg_chunk(mk, q, c):
                        def go():
                            for t in range(c * 8, c * 8 + 8):
                                g = q * SDB + t
                                hf = g & 1
                                nc.tensor.matmul(
                                    seg[hf * CJ:(hf + 1) * CJ, :],
                                    xpb[:, g * CJ:(g + 1) * CJ],
                                    mk[:, t * FD:(t + 1) * FD],
                                    start=(g == hf),
                                    stop=(g == groups - 2 + hf),
                                    skip_group_check=True,
                                    tile_position=(0, hf * CJ))
                        return go

                for q in range(NSUP):
                    s16 = s16pool.tile([P, SDB * FD], F16, tag="s16")
                    if q == 1 and hook is not None:
                        hook()
                    for h in range(4):
                        sp = bigpsum.tile([P, GBATCH * FD], F32, tag="big")
                        for t in range(GBATCH):
                            g = (q * 4 + h) * GBATCH + t
                            nc.tensor.matmul(
                                sp[:, t * FD:(t + 1) * FD],
                                x5b[:, g * P:(g + 1) * P],
                                wdiag16[sl][:], start=True, stop=True)
                        if flushq:
                            flushq.pop(0)()
                        # ACT evacuates scores to fp16 SBUF (k-outer layout)
                        nc.scalar.copy(
                            s16[:, h * GBATCH * FD:(h + 1) * GBATCH * FD],
                            sp[:])
                    s4 = s16[:].rearrange("p (gb k j) -> p gb k j", k=K, j=J)
                    # 3-round pairwise min over k (all 2x packed-16 mode)
                    mv1 = mvpool.tile([P, SDB * 4 * J], F16, tag="mv1")
                    m1 = mv1[:].rearrange("p (gb k j) -> p gb k j", k=4, j=J)
                    nc.vector.tensor_tensor(
                        m1, s4[:, :, 0:4, :], s4[:, :, 4:8, :],
                        mybir.AluOpType.min)
                    mv2 = mvpool.tile([P, SDB * 2 * J], F16, tag="mv2")
                    m2 = mv2[:].rearrange("p (gb k j) -> p gb k j", k=2, j=J)
                    nc.vector.tensor_tensor(
                        m2, m1[:, :, 0:2, :], m1[:, :, 2:4, :],
                        mybir.AluOpType.min)
                    mv3 = mvpool.tile([P, SDB * J], F16, tag="mv3")
                    m3 = mv3[:].rearrange("p (gb j) -> p gb j", j=J)
                    nc.vector.tensor_tensor(
                        m3.unsqueeze(2), m2[:, :, 0:1, :], m2[:, :, 1:2, :],
                        mybir.AluOpType.min)
                    # mask = (s16 <= min) broadcast over k
                    mv_b = bass.AP(
                        tensor=mv3[:].tensor, offset=mv3[:].offset,
                        ap=[mv3[:].ap[0], [J, SDB], [0, K], [1, J]])
                    mk = maskpool.tile([P, SDB * FD], F16, tag="mk")
                    nc.vector.tensor_tensor(
                        mk[:].rearrange("p (gb k j) -> p gb k j", k=K, j=J),
                        s4, mv_b, mybir.AluOpType.is_le)
                    flushq.extend(seg_chunk(mk, q, c) for c in range(4))
                return flushq

            def fold_update_part(sl):
                # ---- fold seg -> S[k, (r,g,b,count)] ----
                if SEG_FP8:
                    # sum the two parity chains, then one 64-row fold
                    # (only one PSUM operand allowed per instruction)
                    nc.scalar.copy(segsum[sl][:], segt[sl][:, 0:FD])
                    nc.vector.tensor_add(
                        segsum[sl][:], segsum[sl][:], segt[sl][:, FD:])
                    nc.vector.tensor_tensor(
                        prod[sl][:], segsum[sl][:], diagk[0:CJ, :],
                        mybir.AluOpType.mult)
                else:
                    nc.vector.tensor_tensor(
                        prod[sl][:], segt[sl][:], diagk[:],
                        mybir.AluOpType.mult)
                nc.vector.tensor_reduce(
                    ext[sl][:],
                    prod[sl][:].rearrange("p (k j) -> p k j", j=J),
                    axis=mybir.AxisListType.X,
                    op=mybir.AluOpType.add)
                S = smallps.tile([K, 4], F32, tag=f"small{sl}")
                nc.tensor.matmul(S[:], ext[sl][:], csel[:],
                                 start=True, stop=True)

                # ---- centroid update ----
                # counts come back scaled by 1.5 in fp8 mode; threshold 0.5
                # keeps n=0 -> mean 0 (masked) and n>=1 exact in both modes.
                nc.vector.tensor_scalar_max(cntc[sl][:], S[:, 3:4], 0.5)
                nc.vector.reciprocal(recip[sl][:], cntc[sl][:])
                # per-partition scale rides the ACT ops (spares the DVE)
                nc.scalar.mul(cmean[sl][:], S[:, 0:D], recip[sl][:])
                nc.vector.tensor_scalar(
                    pos[sl][:], S[:, 3:4], 0.5, None,
                    op0=mybir.AluOpType.is_ge)
                nc.vector.tensor_sub(cdel[sl][:], cmean[sl][:], cent[sl][:])
                nc.scalar.mul(cdel[sl][:], cdel[sl][:], pos[sl][:])
                nc.vector.tensor_add(cent[sl][:], cent[sl][:], cdel[sl][:])

            nc.sync.dma_start(out=cent[0][:], in_=c0_d[0])
            nc.sync.dma_start(out=cent[1][:], in_=c0_d[1])
            dma_image(0)
            dma_image(1)
            weights_part(0)              # pair 0's A weights

            for pair in range(n_img // 2):
                a, b = 2 * pair, 2 * pair + 1
                # prefetch next pair's A image; its x5/xpix slots are
                # unused by this pair. B's slot aliases image a's, so its
                # prefetch is issued after the trip loop below.
                if a + 2 < n_img:
                    dma_image(a + 2)

                xa, pa = x5t[a % NX5], xpixt[a % NXP]
                xb, pb = x5t[b % NX5], xpixt[b % NXP]

                # Fully symmetric software pipeline: each image's
                # fold/update/weights chain is emitted from a hook INSIDE
                # the other image's batch loop (after the carried seg
                # chunks drain), and each image's final super-batch of seg
                # matmuls flushes interleaved with the other image's score
                # bursts. On the last trip, A's result ships early and the
                # NEXT pair's A-centroids + weight chain are built under
                # B's final batch loop.
                def hook_b0():
                    weights_part(1)

                def hook_a():
                    fold_update_part(1)
                    weights_part(1)

                def hook_b():
                    fold_update_part(0)
                    weights_part(0)

                def hook_b_last():
                    fold_update_part(0)
                    nc.sync.dma_start(out=out_d[a], in_=cent[0][:])
                    if a + 2 < n_img:
                        nc.sync.dma_start(out=cent[0][:], in_=c0_d[a + 2])
                        weights_part(0)   # next pair's A weights

                carry = batches_part(0, xa, pa, hook=hook_b0)
                carry = batches_part(1, xb, pb, carry=carry, hook=hook_b)
                for t in range(1, iters):
                    carry = batches_part(0, xa, pa, carry=carry, hook=hook_a)
                    carry = batches_part(
                        1, xb, pb, carry=carry,
                        hook=hook_b if t < iters - 1 else hook_b_last)
                for chunk in carry:
                    chunk()
                fold_update_part(1)

                nc.sync.dma_start(out=out_d[b], in_=cent[1][:])
                if b + 2 < n_img:
                    nc.sync.dma_start(out=cent[1][:], in_=c0_d[b + 2])
                    dma_image(b + 2)

    nc.finalize()
    return nc


# ----------------------------------------------------------------------------
# host-side layouts


def host_layouts(pixels):
    """pixels [B, N, 3] f32 -> (x5 [B, 128, 16384] f16, xpix fp8/fp16).

    x5[(b,j), (g,p)]: bands 0-2 / 3-5 = x_rgb fp16 (hi/lo share data),
    band 6 = 1.0, band 7 = |x|^2 fp16, for pixel g*J*P + j*P + p.
    xpix fp8 mode: [p, (g, s, c, j)], s=0 zeros (pairs with the always-zero
    low mask byte), s=1 = fp8 pixel values; c in {r,g,b,1}.
    """
    b = pixels.shape[0]
    g = GROUPS
    v = pixels.reshape(b, g, J, P, D)
    rgb = np.ascontiguousarray(
        v.transpose(0, 4, 2, 1, 3).reshape(b, D * J, g * P)).astype(np.float16)
    xsq = (pixels.astype(np.float32) ** 2).sum(-1).astype(np.float16)
    xsqr = np.ascontiguousarray(
        xsq.reshape(b, g, J, P).transpose(0, 2, 1, 3).reshape(b, J, g * P))
    x5 = np.empty((b, NB * J, g * P), np.float16)
    x5[:, 0:48] = rgb
    x5[:, 48:96] = rgb
    x5[:, 96:112] = np.float16(1.0)
    x5[:, 112:128] = xsqr
    if SEG_FP8:
        xp = np.zeros((b, P, g, 2, 4, J), ml_dtypes.float8_e4m3)
        xp[..., 1, 0:3, :] = v.transpose(0, 3, 1, 4, 2).astype(
            ml_dtypes.float8_e4m3)  # b p g c j
        xp[..., 1, 3, :] = ml_dtypes.float8_e4m3(1.0)
        xpix = np.ascontiguousarray(xp.reshape(b, P, g * 2 * CJ))
    else:
        xp = np.empty((b, P, g, 4, J), np.float16)
        xp[..., 0:3, :] = v.transpose(0, 3, 1, 4, 2).astype(np.float16)
        xp[..., 3, :] = np.float16(1.0)
        xpix = np.ascontiguousarray(xp.reshape(b, P, g * CJ))
    return x5, xpix


def host_constants():
    diagk = np.zeros((NB * J, FD), np.float32)
    for bnd in range(NB):
        for j in range(J):
            for k in range(K):
                diagk[bnd * J + j, k * J + j] = 1.0
    if SEG_FP8:
        csel = np.zeros((CJ, 4), np.float32)
        for c in range(4):
            for j in range(J):
                csel[c * J + j, c] = 1.0
    else:
        csel = np.zeros((2 * CJ, 4), np.float32)
        for h in range(2):
            for c in range(4):
                for j in range(J):
                    csel[h * CJ + c * J + j, c] = 1.0
    # bca: wt5 rows (-2cx,-2cy,-2cz, cc, 1) -> bands (0,1,2, 6, 7)
    # bcb: wlo rows (lox,loy,loz, junk) -> bands (3,4,5, -)
    bca = np.zeros((5, NB * J), np.float32)
    bcb = np.zeros((4, NB * J), np.float32)
    for j in range(J):
        for r, bnd in enumerate((0, 1, 2, 6, 7)):
            bca[r, bnd * J + j] = 1.0
        for r, bnd in enumerate((3, 4, 5)):
            bcb[r, bnd * J + j] = 1.0
    ident = np.eye(K, dtype=np.float32)
    return diagk, csel, bca, bcb, ident


_NC_CACHE = {}
TRACE = False
LAST_RESULTS = None


def _get_nc(n_img, iters, groups):
    key = (n_img, iters, groups)
    if key not in _NC_CACHE:
        _NC_CACHE[key] = build_kernel(n_img, iters, groups)
    return _NC_CACHE[key]


def kernel(inputs: np.ndarray) -> np.ndarray:
    x = np.ascontiguousarray(np.asarray(inputs, dtype=np.float32))
    assert x.shape == (B, H, W, D), x.shape
    pixels = x.reshape(B, N, D)

    perm8 = np.array(PERM8, dtype=np.int64)
    cent0 = np.take_along_axis(
        pixels, perm8[:, :, None].repeat(D, axis=2), axis=1
    ).astype(np.float32)

    x5, xpix = host_layouts(pixels)
    diagk, csel, bca, bcb, ident = host_constants()
    nc = _get_nc(IMG_PER_CORE, ITERS, GROUPS)

    in_maps = []
    for c in range(NCORES):
        sl = slice(c * IMG_PER_CORE, (c + 1) * IMG_PER_CORE)
        in_maps.append({
            "x5": np.ascontiguousarray(x5[sl]),
            "xpix": np.ascontiguousarray(xpix[sl]),
            "cent0": np.ascontiguousarray(cent0[sl]),
            "diagk": diagk,
            "csel": csel,
            "bca": bca,
            "bcb": bcb,
            "ident": ident,
        })

    global LAST_RESULTS
    try:
        res = run_bass_kernel_spmd(nc, in_maps, core_ids=list(range(NCORES)),
                                   trace=TRACE)
    except Exception:
        if not TRACE:
            raise
        res = run_bass_kernel_spmd(nc, in_maps, core_ids=list(range(NCORES)))
    LAST_RESULTS = res
    outs = [r["cent_out"].reshape(IMG_PER_CORE, K * D) for r in res.results]
    return np.concatenate(outs, axis=0).astype(np.float32)


if __name__ == "__main__":
    rs = np.random.RandomState(0)
    x = rs.random_sample((B, H, W, D)).astype(np.float32)
    out = kernel(inputs=x)
    print("out shape", out.shape, out.dtype)
    print(out[0])


# revision 24
# speedup vs baseline: 1.3061x; 1.3050x over previous
"""Trainium2 Bass kernel for nn_ColorExtractor (per-image k-means, K=8, 10 iters).

Contract: kernel(**inputs) takes FULL inputs ([64, 512, 512, 3] f32), returns
FULL output ([64, 24] f32), batch sharded over 8 NeuronCores (8 images/core).

v3 design (vs v2):
  seg     segment-sum matmuls run in fp8 DoubleRow mode at 2x PE rate: the
          fp16 0/1 mask buffer is bitcast to fp8e4 pairs (lo byte always
          0x00 -> 0.0, hi byte 0x3C -> 1.5) and fed as the two DoubleRow
          contraction subtiles; xpix is stored fp8 with a zeroed partner
          subtile. Sums come out scaled by exactly 1.5, which cancels in
          means; the count threshold drops to 0.5 (n=0 -> masked anyway).
  argmin  DVE work batched at 32-group granularity (half the instruction
          dispatch of v2); min tree + is_le unchanged otherwise.
  weights w8 = [-2c | sum c^2] built on ACT (mul + Square-with-accum),
          freeing DVE cycles.

Initial centroids replicate jax.random.permutation(key, N)[:8] via the
precomputed PERM8 table (numpy threefry port, verified bit-exact).
"""

import numpy as np

try:                      # only needed for the (disabled) fp8 seg path
    import ml_dtypes
except ImportError:       # pragma: no cover
    ml_dtypes = None

import concourse.bacc as bacc
import concourse.bass as bass
import concourse.tile as tile
from concourse import mybir
from concourse.bass_utils import run_bass_kernel_spmd

# ----------------------------------------------------------------------------
# problem constants (hardcoded per contract)
B = 64            # total images
NCORES = 8
IMG_PER_CORE = B // NCORES
H = W = 512
N = H * W         # pixels per image: 262144
K = 8             # clusters
ITERS = 10
D = 3

# device tiling
P = 128           # pixels per chunk
J = 16            # chunks per matmul group (block-diag j packing)
NB = 8            # contraction bands: r,g,b, r,g,b(lo), ones, |x|^2
GROUPS = N // (J * P)    # 128 groups per image
GBATCH = 8        # groups per PSUM batch
NFILL = 2         # PSUM fills per DVE super-batch
SDB = NFILL * GBATCH     # groups per DVE super-batch
NSUP = GROUPS // SDB
FD = K * J        # 128: free dim of scores/seg matmuls, cols = (k, j)
CJ = 4 * J        # 64: xpix cols per group, (c in {r,g,b,1}, j)

F32 = mybir.dt.float32
F16 = mybir.dt.float16
F8 = mybir.dt.float8e4

# fp8 DoubleRow segment sums (mask byte-alias trick): measured rel err
# 1.98e-2 vs the 2e-2 gate (fp8 xpix quantization excites trajectory chaos
# in k-means) and the DR matmuls serialize LDWEIGHTS. Keep the fp16 path.
SEG_FP8 = False

# ----------------------------------------------------------------------------
# numpy threefry port (verified bit-exact vs jax 0.8 threefry2x32 impl)
_U32 = np.uint32


def _rotl(x, d):
    d = _U32(d)
    return (x << d) | (x >> _U32(32 - d))


def _threefry2x32(k1, k2, x1, x2):
    with np.errstate(over="ignore"):
        ks0, ks1 = _U32(k1), _U32(k2)
        ks2 = _U32(ks0 ^ ks1 ^ _U32(0x1BD11BDA))
        x = [(x1 + ks0).astype(_U32), (x2 + ks1).astype(_U32)]

        def rounds(rots, ka, kb, inc):
            for r in rots:
                x[0] = (x[0] + x[1]).astype(_U32)
                x[1] = _rotl(x[1], r)
                x[1] = x[0] ^ x[1]
            x[0] = (x[0] + ka).astype(_U32)
            x[1] = (x[1] + kb + _U32(inc)).astype(_U32)

        rounds((13, 15, 26, 6), ks1, ks2, 1)
        rounds((17, 29, 16, 24), ks2, ks0, 2)
        rounds((13, 15, 26, 6), ks0, ks1, 3)
        rounds((17, 29, 16, 24), ks1, ks2, 4)
        rounds((13, 15, 26, 6), ks2, ks0, 5)
    return x[0], x[1]


def _tf_split(key, num):
    i = np.arange(num, dtype=np.uint64)
    b1, b2 = _threefry2x32(key[0], key[1],
                           (i >> np.uint64(32)).astype(_U32), i.astype(_U32))
    return np.stack([b1, b2], axis=1)


def _tf_bits(key, n):
    i = np.arange(n, dtype=np.uint64)
    b1, b2 = _threefry2x32(key[0], key[1],
                           (i >> np.uint64(32)).astype(_U32), i.astype(_U32))
    return b1 ^ b2


def jax_permutation_indices(seed, batch, n):
    """perm[b] = jax.random.permutation(split(key(seed), batch)[b], n)."""
    keys = _tf_split(np.array([0, seed], _U32), batch)
    num_rounds = int(np.ceil(3 * np.log(max(1, n)) / np.log(2**32 - 1)))
    perms = []
    for b in range(batch):
        x = np.arange(n)
        k = keys[b]
        for _ in range(num_rounds):
            ks = _tf_split(k, 2)
            k = ks[0]
            sort_keys = _tf_bits(ks[1], n)
            x = x[np.argsort(sort_keys, kind="stable")]
        perms.append(x[:K])
    return np.stack(perms)  # [batch, K]


# Precomputed jax.random.permutation(split(key(42), 64)[b], N)[:8] indices
# (input-independent; verified against the threefry port above).
PERM8 = (
    (121373, 128858, 64733, 199519, 198377, 234239, 198325, 209106),
    (73520, 236184, 209288, 97370, 64322, 228694, 126128, 72161),
    (143944, 27877, 97040, 2149, 10994, 109181, 179954, 54887),
    (147613, 8773, 54262, 44295, 29289, 11407, 31612, 133442),
    (206432, 166428, 5023, 212109, 16365, 21194, 249053, 195143),
    (13257, 110295, 84080, 119151, 246640, 69532, 130091, 105945),
    (14760, 174397, 198857, 826, 140745, 258776, 214608, 163989),
    (184593, 240934, 160738, 23779, 43199, 47433, 94941, 50416),
    (4386, 21260, 129661, 125128, 50701, 200388, 254109, 44816),
    (203980, 230711, 102351, 31296, 161690, 63692, 194032, 60281),
    (170168, 75997, 12072, 137876, 34146, 48636, 181597, 67859),
    (218987, 48148, 224774, 27163, 85280, 163529, 107708, 238871),
    (152153, 120028, 50368, 168498, 254864, 185234, 259971, 5221),
    (126051, 57270, 7614, 194865, 246341, 83824, 226962, 115962),
    (68603, 18235, 201699, 6558, 217064, 74053, 140307, 29320),
    (212222, 174163, 63891, 131714, 260991, 125525, 109871, 254552),
    (208133, 37817, 108871, 236086, 230829, 224735, 197202, 126789),
    (36220, 183667, 173531, 231574, 63007, 23270, 242256, 172824),
    (226174, 181177, 45094, 10219, 172720, 14537, 122494, 27364),
    (19288, 1130, 162371, 12239, 106820, 190833, 228451, 33845),
    (420, 256427, 250298, 234965, 137965, 33886, 192615, 137263),
    (30426, 206099, 1480, 169907, 122972, 5299, 178194, 116853),
    (38366, 252943, 119579, 233642, 99176, 152381, 1818, 246484),
    (49412, 124354, 252000, 221213, 103625, 2726, 153653, 148581),
    (82319, 1626, 107383, 158105, 81846, 13120, 1198, 193305),
    (44406, 239081, 240884, 84662, 7763, 52627, 182256, 187716),
    (185632, 105456, 212756, 173585, 81328, 74972, 128159, 45046),
    (104599, 7215, 61087, 26573, 59314, 48591, 945, 28553),
    (127710, 94893, 75476, 221733, 184125, 96685, 172243, 242612),
    (42647, 29769, 148111, 39823, 193859, 57502, 144317, 214559),
    (780, 145567, 79710, 226978, 2835, 160638, 8378, 24523),
    (161231, 246284, 44873, 150516, 114149, 68239, 117811, 141424),
    (31461, 110744, 232951, 16033, 179041, 106854, 47200, 63782),
    (255322, 241469, 248608, 95048, 170033, 253394, 261582, 181885),
    (63034, 5, 212309, 79222, 1841, 237107, 261430, 22474),
    (203738, 21095, 211942, 6233, 26825, 175918, 126433, 89713),
    (57893, 173681, 13566, 126980, 140303, 73406, 105028, 86705),
    (15800, 76765, 217596, 184873, 201602, 112166, 76158, 112065),
    (110522, 160113, 18684, 10469, 166599, 145226, 99589, 158310),
    (214726, 131223, 109288, 126812, 105792, 167086, 256918, 18441),
    (164736, 182565, 35066, 89660, 98586, 130539, 202194, 16684),
    (24903, 25959, 122313, 26525, 105627, 87218, 23062, 109362),
    (67552, 140412, 247510, 126439, 184322, 171107, 87397, 165128),
    (211326, 162921, 221946, 131793, 156106, 253917, 2345, 133918),
    (219591, 25610, 154884, 239521, 173390, 39973, 114213, 162088),
    (69694, 51180, 74827, 176121, 132947, 148345, 15083, 196459),
    (229624, 100015, 196100, 105569, 78527, 72176, 225549, 208691),
    (158498, 42753, 240006, 246065, 213196, 49877, 129372, 244273),
    (51001, 229538, 39704, 237637, 58774, 83576, 211231, 135814),
    (173630, 162748, 219633, 240928, 8298, 5311, 113776, 113251),
    (64061, 16436, 138070, 47525, 57016, 229742, 159929, 228539),
    (73108, 34503, 7538, 165920, 68681, 114191, 193009, 48042),
    (2842, 97501, 29489, 248778, 176907, 223147, 54452, 11731),
    (224345, 79068, 183290, 239324, 14912, 169078, 122283, 32914),
    (95340, 11646, 45163, 48387, 78062, 60978, 227735, 162106),
    (258986, 131616, 85766, 51383, 132449, 213013, 150516, 231609),
    (65332, 246689, 206208, 181886, 235636, 139183, 132468, 6602),
    (6778, 179487, 58159, 114248, 26277, 180706, 54969, 240497),
    (15413, 19595, 73952, 219244, 68813, 152629, 243501, 175077),
    (208668, 251169, 186627, 98857, 78225, 13125, 12392, 28954),
    (81754, 93281, 49839, 112579, 166016, 88571, 91558, 20863),
    (108264, 245898, 72992, 168504, 68263, 195879, 27596, 23576),
    (44918, 166098, 212537, 239555, 231283, 94408, 203172, 18701),
    (113563, 111669, 16481, 161974, 22111, 116384, 31096, 252828),
)


# ----------------------------------------------------------------------------
# device kernel builder


def build_kernel(n_img=IMG_PER_CORE, iters=ITERS, groups=GROUPS):
    nc = bacc.Bacc("TRN2", target_bir_lowering=False)

    XPW = 2 * CJ if SEG_FP8 else CJ    # xpix cols per group on device
    xpix_dt = F8 if SEG_FP8 else F16

    x5_d = nc.dram_tensor("x5", [n_img, NB * J, groups * P], F16,
                          kind="ExternalInput")
    xpix_d = nc.dram_tensor("xpix", [n_img, P, groups * XPW], xpix_dt,
                            kind="ExternalInput")
    SEGR_ = CJ if SEG_FP8 else 2 * CJ
    c0_d = nc.dram_tensor("cent0", [n_img, K, D], F32, kind="ExternalInput")
    diagk_d = nc.dram_tensor("diagk", [NB * J, FD], F32, kind="ExternalInput")
    csel_d = nc.dram_tensor("csel", [SEGR_, 4], F32, kind="ExternalInput")
    bca_d = nc.dram_tensor("bca", [5, NB * J], F32, kind="ExternalInput")
    bcb_d = nc.dram_tensor("bcb", [4, NB * J], F32, kind="ExternalInput")
    ident_d = nc.dram_tensor("ident", [K, K], F32, kind="ExternalInput")
    out_d = nc.dram_tensor("cent_out", [n_img, K, D], F32, kind="ExternalOutput")

    with tile.TileContext(nc) as tc:
        with (
            tc.tile_pool(name="singles", bufs=1) as singles,
            tc.tile_pool(name="s16p", bufs=3) as s16pool,
            tc.tile_pool(name="maskp", bufs=3) as maskpool,
            tc.tile_pool(name="mvp", bufs=1) as mvpool,
            tc.tile_pool(name="bigpsum", bufs=2, space="PSUM") as bigpsum,
            tc.tile_pool(name="segpsum", bufs=1, space="PSUM") as segpsum,
            tc.tile_pool(name="smallpsum", bufs=1, space="PSUM") as smallps,
        ):
            # --- constants ---
            diagk = singles.tile([NB * J, FD], F32, tag="diagk")
            nc.sync.dma_start(out=diagk[:], in_=diagk_d[:])
            diagk16 = singles.tile([NB * J, FD], F16, tag="diagk16")
            nc.scalar.copy(diagk16[:], diagk[:])
            csel = singles.tile([SEGR_, 4], F32, tag="csel")
            nc.sync.dma_start(out=csel[:], in_=csel_d[:])
            bca = singles.tile([5, NB * J], F32, tag="bca")
            nc.sync.dma_start(out=bca[:], in_=bca_d[:])
            bcb = singles.tile([4, NB * J], F32, tag="bcb")
            nc.sync.dma_start(out=bcb[:], in_=bcb_d[:])
            ident = singles.tile([K, K], F32, tag="ident")
            nc.sync.dma_start(out=ident[:], in_=ident_d[:])

            # --- persistent state ---
            # x5 lives in a 3-slot ring; xpix in a 4-slot ring. Two images
            # (a pair) are interleaved inside the trip loop so each image's
            # serial fold/update/weight chain hides under the other's batch
            # loop; the next pair prefetches during the current one.
            NX5 = 3
            x5t = [singles.tile([NB * J, groups * P], F16, tag=f"x5_{i}",
                                name=f"x5_{i}")
                   for i in range(NX5)]
            NXP = 3
            xpixt = [singles.tile([P, groups * XPW], xpix_dt, tag=f"xp_{i}",
                                  name=f"xp_{i}")
                     for i in range(NXP)]
            # per-pair-slot state (index 0 = image A, 1 = image B)
            cent = [singles.tile([K, D], F32, tag=f"cent{i}", name=f"cent{i}")
                    for i in range(2)]
            w8 = [singles.tile([K, 5], F32, tag=f"w8_{i}", name=f"w8_{i}")
                  for i in range(2)]
            for t in w8:
                nc.vector.memset(t[:, 4:5], 1.0)  # |x|^2 band weight
            csq = [singles.tile([K, D], F32, tag=f"csq{i}", name=f"csq{i}")
                   for i in range(2)]
            wt5 = [singles.tile([5, K], F32, tag=f"wt5_{i}", name=f"wt5_{i}")
                   for i in range(2)]
            whi16 = [singles.tile([4, K], F16, tag=f"whi{i}", name=f"whi{i}")
                     for i in range(2)]
            wlo = [singles.tile([4, K], F32, tag=f"wlo{i}", name=f"wlo{i}")
                   for i in range(2)]
            wrep = [singles.tile([NB * J, K], F16, tag=f"wrp{i}", name=f"wrp{i}")
                    for i in range(2)]
            wdiag16 = [singles.tile([NB * J, FD], F16, tag=f"wd{i}",
                                    name=f"wd{i}")
                       for i in range(2)]
            SEGR = CJ if SEG_FP8 else 2 * CJ   # fold row count
            prod = [singles.tile([SEGR, FD], F32, tag=f"prod{i}",
                                 name=f"prod{i}")
                    for i in range(2)]
            ext = [singles.tile([SEGR, K], F32, tag=f"ext{i}", name=f"ext{i}")
                   for i in range(2)]
            cntc = [singles.tile([K, 1], F32, tag=f"cnt{i}", name=f"cnt{i}")
                    for i in range(2)]
            recip = [singles.tile([K, 1], F32, tag=f"rcp{i}", name=f"rcp{i}")
                     for i in range(2)]
            pos = [singles.tile([K, 1], F32, tag=f"pos{i}", name=f"pos{i}")
                   for i in range(2)]
            cmean = [singles.tile([K, D], F32, tag=f"cm{i}", name=f"cm{i}")
                     for i in range(2)]
            cdel = [singles.tile([K, D], F32, tag=f"cd{i}", name=f"cd{i}")
                    for i in range(2)]

            NQ = 4

            def dma_image(img):
                x5b, xpb = x5t[img % NX5], xpixt[img % NXP]
                w = groups * P // NQ
                for q in range(NQ):
                    nc.sync.dma_start(
                        out=x5b[:, q * w:(q + 1) * w],
                        in_=x5_d[img][:, q * w:(q + 1) * w])
                w2 = groups * XPW // 2
                for q in range(2):
                    nc.sync.dma_start(
                        out=xpb[:, q * w2:(q + 1) * w2],
                        in_=xpix_d[img][:, q * w2:(q + 1) * w2])

            # persistent PSUM accumulators for the two in-flight images.
            # fp8 DoubleRow dst must sit at partition base 0, so the two
            # group-parity accumulation chains live side by side in the free
            # dim ([64, 2*FD]) instead of stacked on partitions.
            if SEG_FP8:
                segt = [segpsum.tile([CJ, 2 * FD], F32, tag=f"seg{i}",
                                     name=f"seg{i}")
                        for i in range(2)]
                segsum = [singles.tile([CJ, FD], F32, tag=f"ss{i}",
                                       name=f"ss{i}")
                          for i in range(2)]
            else:
                segt = [segpsum.tile([2 * CJ, FD], F32, tag=f"seg{i}",
                                     name=f"seg{i}")
                        for i in range(2)]

            def weights_part(sl):
                # ---- weights from centroids ----
                # w8 = [-2c | sum(c^2) | 1]; built on ACT to spare the DVE
                nc.scalar.mul(w8[sl][:, 0:D], cent[sl][:], -2.0)
                nc.scalar.activation(
                    csq[sl][:], cent[sl][:],
                    mybir.ActivationFunctionType.Square,
                    accum_out=w8[sl][:, 3:4])
                wtP = smallps.tile([5, K], F32, tag=f"small{sl}")
                nc.tensor.transpose(wtP[:], w8[sl][:], ident[:])
                nc.scalar.copy(wt5[sl][:], wtP[:])
                # lo-correction rows: wlo = wt - fp16(wt) for the -2c rows
                nc.scalar.copy(whi16[sl][:], wt5[sl][0:4, :])
                nc.vector.tensor_sub(wlo[sl][:], wt5[sl][0:4, :], whi16[sl][:])
                # wrep[(b,j), k] = per-band weight: bands 0-2 <- -2c,
                # 3-5 <- lo(-2c), 6 <- |c|^2, 7 <- 1 (two accumulated MMs
                # with constant selector matrices; no partition shifts)
                wrepP = smallps.tile([NB * J, K], F32, tag=f"small{sl}")
                nc.tensor.matmul(wrepP[:], bca[:], wt5[sl][:],
                                 start=True, stop=False)
                nc.tensor.matmul(wrepP[:], bcb[:], wlo[sl][:],
                                 start=False, stop=True)
                nc.scalar.copy(wrep[sl][:], wrepP[:])
                # wdiag16[(b,j), (k,j')] = wrep[(b,j), k] * 1[j==j']
                # (all-fp16 operands keep the DVE in 2x packed mode; the
                # fp16 rounding of wrep matches wdiag16's own rounding)
                wrep_b = bass.AP(
                    tensor=wrep[sl][:].tensor, offset=wrep[sl][:].offset,
                    ap=[wrep[sl][:].ap[0], [1, K], [0, J]])
                nc.vector.tensor_tensor(
                    wdiag16[sl][:].rearrange("p (k j) -> p k j", j=J),
                    diagk16[:].rearrange("p (k j) -> p k j", j=J),
                    wrep_b, mybir.AluOpType.mult)

            def batches_part(sl, x5b, xpb, carry=None, hook=None):
                # ---- main loop over super-batches. Seg matmuls trail their
                # mask by one super-batch, flushed in 8-MM chunks between
                # score bursts so the PE interleaves scores and seg work and
                # copies are never head-of-line blocked. The final super-
                # batch's chunks are RETURNED and flush inside the OTHER
                # image's batch loop (cross-image carry); `hook` emits that
                # image's fold/weights chain once the carry has drained. ----
                seg = segt[sl]
                flushq = list(carry) if carry else []

                if SEG_FP8:
                    def seg_chunk(mk, q, c):
                        def go():
                            mk8 = mk[:].bitcast(F8).rearrange(
                                "p (n two) -> p two n", two=2)
                            for t in range(c * 8, c * 8 + 8):
                                g = q * SDB + t
                                hf = g & 1
                                xp8 = xpb[:, g * XPW:(g + 1) * XPW].rearrange(
                                    "p (two c) -> p two c", two=2)
                                nc.tensor.matmul(
                                    seg[:, hf * FD:(hf + 1) * FD],
                                    xp8,
                                    mk8[:, :, t * FD:(t + 1) * FD],
                                    perf_mode=mybir.MatmulPerfMode.DoubleRow,
                                    start=(g == hf),
                                    stop=(g == groups - 2 + hf),
                                    skip_group_check=True)
                        return go
                else:
                    def seg_chunk(mk, q, c):
                        def go():
                            for t in range(c * 8, c * 8 + 8):
                                g = q * SDB + t
                                hf = g & 1
                                nc.tensor.matmul(
                                    seg[hf * CJ:(hf + 1) * CJ, :],
                                    xpb[:, g * CJ:(g + 1) * CJ],
                                    mk[:, t * FD:(t + 1) * FD],
                                    start=(g == hf),
                                    stop=(g == groups - 2 + hf),
                                    skip_group_check=True,
                                    tile_position=(0, hf * CJ))
                        return go

                for q in range(NSUP):
                    s16 = s16pool.tile([P, SDB * FD], F16, tag="s16")
                    if q == 1 and hook is not None:
                        hook()
                    for h in range(NFILL):
                        sp = bigpsum.tile([P, GBATCH * FD], F32, tag="big")
                        for t in range(GBATCH):
                            g = (q * NFILL + h) * GBATCH + t
                            nc.tensor.matmul(
                                sp[:, t * FD:(t + 1) * FD],
                                x5b[:, g * P:(g + 1) * P],
                                wdiag16[sl][:], start=True, stop=True)
                        if flushq:
                            flushq.pop(0)()
                        # ACT evacuates scores to fp16 SBUF (k-outer layout)
                        nc.scalar.copy(
                            s16[:, h * GBATCH * FD:(h + 1) * GBATCH * FD],
                            sp[:])
                    s4 = s16[:].rearrange("p (gb k j) -> p gb k j", k=K, j=J)
                    # 3-round pairwise min over k (all 2x packed-16 mode)
                    mv1 = mvpool.tile([P, SDB * 4 * J], F16, tag="mv1")
                    m1 = mv1[:].rearrange("p (gb k j) -> p gb k j", k=4, j=J)
                    nc.vector.tensor_tensor(
                        m1, s4[:, :, 0:4, :], s4[:, :, 4:8, :],
                        mybir.AluOpType.min)
                    mv2 = mvpool.tile([P, SDB * 2 * J], F16, tag="mv2")
                    m2 = mv2[:].rearrange("p (gb k j) -> p gb k j", k=2, j=J)
                    nc.vector.tensor_tensor(
                        m2, m1[:, :, 0:2, :], m1[:, :, 2:4, :],
                        mybir.AluOpType.min)
                    mv3 = mvpool.tile([P, SDB * J], F16, tag="mv3")
                    m3 = mv3[:].rearrange("p (gb j) -> p gb j", j=J)
                    nc.vector.tensor_tensor(
                        m3.unsqueeze(2), m2[:, :, 0:1, :], m2[:, :, 1:2, :],
                        mybir.AluOpType.min)
                    # mask = (s16 <= min) broadcast over k
                    mv_b = bass.AP(
                        tensor=mv3[:].tensor, offset=mv3[:].offset,
                        ap=[mv3[:].ap[0], [J, SDB], [0, K], [1, J]])
                    mk = maskpool.tile([P, SDB * FD], F16, tag="mk")
                    nc.vector.tensor_tensor(
                        mk[:].rearrange("p (gb k j) -> p gb k j", k=K, j=J),
                        s4, mv_b, mybir.AluOpType.is_le)
                    flushq.extend(seg_chunk(mk, q, c) for c in range(SDB // 8))
                return flushq

            def fold_update_part(sl):
                # ---- fold seg -> S[k, (r,g,b,count)] ----
                if SEG_FP8:
                    # sum the two parity chains, then one 64-row fold
                    # (only one PSUM operand allowed per instruction)
                    nc.scalar.copy(segsum[sl][:], segt[sl][:, 0:FD])
                    nc.vector.tensor_add(
                        segsum[sl][:], segsum[sl][:], segt[sl][:, FD:])
                    nc.vector.tensor_tensor(
                        prod[sl][:], segsum[sl][:], diagk[0:CJ, :],
                        mybir.AluOpType.mult)
                else:
                    nc.vector.tensor_tensor(
                        prod[sl][:], segt[sl][:], diagk[:],
                        mybir.AluOpType.mult)
                nc.vector.tensor_reduce(
                    ext[sl][:],
                    prod[sl][:].rearrange("p (k j) -> p k j", j=J),
                    axis=mybir.AxisListType.X,
                    op=mybir.AluOpType.add)
                S = smallps.tile([K, 4], F32, tag=f"small{sl}")
                nc.tensor.matmul(S[:], ext[sl][:], csel[:],
                                 start=True, stop=True)

                # ---- centroid update ----
                # counts come back scaled by 1.5 in fp8 mode; threshold 0.5
                # keeps n=0 -> mean 0 (masked) and n>=1 exact in both modes.
                nc.vector.tensor_scalar_max(cntc[sl][:], S[:, 3:4], 0.5)
                nc.vector.reciprocal(recip[sl][:], cntc[sl][:])
                # per-partition scale rides the ACT ops (spares the DVE)
                nc.scalar.mul(cmean[sl][:], S[:, 0:D], recip[sl][:])
                nc.vector.tensor_scalar(
                    pos[sl][:], S[:, 3:4], 0.5, None,
                    op0=mybir.AluOpType.is_ge)
                nc.vector.tensor_sub(cdel[sl][:], cmean[sl][:], cent[sl][:])
                nc.scalar.mul(cdel[sl][:], cdel[sl][:], pos[sl][:])
                nc.vector.tensor_add(cent[sl][:], cent[sl][:], cdel[sl][:])

            nc.sync.dma_start(out=cent[0][:], in_=c0_d[0])
            nc.sync.dma_start(out=cent[1][:], in_=c0_d[1])
            dma_image(0)
            dma_image(1)
            weights_part(0)              # pair 0's A weights

            for pair in range(n_img // 2):
                a, b = 2 * pair, 2 * pair + 1
                # prefetch next pair's A image; its x5/xpix slots are
                # unused by this pair. B's slot aliases image a's, so its
                # prefetch is issued after the trip loop below.
                if a + 2 < n_img:
                    dma_image(a + 2)

                xa, pa = x5t[a % NX5], xpixt[a % NXP]
                xb, pb = x5t[b % NX5], xpixt[b % NXP]

                # Fully symmetric software pipeline: each image's
                # fold/update/weights chain is emitted from a hook INSIDE
                # the other image's batch loop (after the carried seg
                # chunks drain), and each image's final super-batch of seg
                # matmuls flushes interleaved with the other image's score
                # bursts. On the last trip, A's result ships early and the
                # NEXT pair's A-centroids + weight chain are built under
                # B's final batch loop.
                def hook_b0():
                    weights_part(1)

                def hook_a():
                    fold_update_part(1)
                    weights_part(1)

                def hook_b():
                    fold_update_part(0)
                    weights_part(0)

                def hook_b_last():
                    fold_update_part(0)
                    nc.sync.dma_start(out=out_d[a], in_=cent[0][:])
                    if a + 2 < n_img:
                        nc.sync.dma_start(out=cent[0][:], in_=c0_d[a + 2])
                        weights_part(0)   # next pair's A weights

                carry = batches_part(0, xa, pa, hook=hook_b0)
                carry = batches_part(1, xb, pb, carry=carry, hook=hook_b)
                for t in range(1, iters):
                    carry = batches_part(0, xa, pa, carry=carry, hook=hook_a)
                    carry = batches_part(
                        1, xb, pb, carry=carry,
                        hook=hook_b if t < iters - 1 else hook_b_last)
                for chunk in carry:
                    chunk()
                fold_update_part(1)

                nc.sync.dma_start(out=out_d[b], in_=cent[1][:])
                if b + 2 < n_img:
                    nc.sync.dma_start(out=cent[1][:], in_=c0_d[b + 2])
                    dma_image(b + 2)

    nc.finalize()
    return nc


# ----------------------------------------------------------------------------
# host-side layouts


def host_layouts(pixels):
    """pixels [B, N, 3] f32 -> (x5 [B, 128, 16384] f16, xpix fp8/fp16).

    x5[(b,j), (g,p)]: bands 0-2 / 3-5 = x_rgb fp16 (hi/lo share data),
    band 6 = 1.0, band 7 = |x|^2 fp16, for pixel g*J*P + j*P + p.
    xpix fp8 mode: [p, (g, s, c, j)], s=0 zeros (pairs with the always-zero
    low mask byte), s=1 = fp8 pixel values; c in {r,g,b,1}.
    """
    b = pixels.shape[0]
    g = GROUPS
    v = pixels.reshape(b, g, J, P, D)
    rgb = np.ascontiguousarray(
        v.transpose(0, 4, 2, 1, 3).reshape(b, D * J, g * P)).astype(np.float16)
    xsq = (pixels.astype(np.float32) ** 2).sum(-1).astype(np.float16)
    xsqr = np.ascontiguousarray(
        xsq.reshape(b, g, J, P).transpose(0, 2, 1, 3).reshape(b, J, g * P))
    x5 = np.empty((b, NB * J, g * P), np.float16)
    x5[:, 0:48] = rgb
    x5[:, 48:96] = rgb
    x5[:, 96:112] = np.float16(1.0)
    x5[:, 112:128] = xsqr
    if SEG_FP8:
        xp = np.zeros((b, P, g, 2, 4, J), ml_dtypes.float8_e4m3)
        xp[..., 1, 0:3, :] = v.transpose(0, 3, 1, 4, 2).astype(
            ml_dtypes.float8_e4m3)  # b p g c j
        xp[..., 1, 3, :] = ml_dtypes.float8_e4m3(1.0)
        xpix = np.ascontiguousarray(xp.reshape(b, P, g * 2 * CJ))
    else:
        xp = np.empty((b, P, g, 4, J), np.float16)
        xp[..., 0:3, :] = v.transpose(0, 3, 1, 4, 2).astype(np.float16)
        xp[..., 3, :] = np.float16(1.0)
        xpix = np.ascontiguousarray(xp.reshape(b, P, g * CJ))
    return x5, xpix


def host_constants():
    diagk = np.zeros((NB * J, FD), np.float32)
    for bnd in range(NB):
        for j in range(J):
            for k in range(K):
                diagk[bnd * J + j, k * J + j] = 1.0
    if SEG_FP8:
        csel = np.zeros((CJ, 4), np.float32)
        for c in range(4):
            for j in range(J):
                csel[c * J + j, c] = 1.0
    else:
        csel = np.zeros((2 * CJ, 4), np.float32)
        for h in range(2):
            for c in range(4):
                for j in range(J):
                    csel[h * CJ + c * J + j, c] = 1.0
    # bca: wt5 rows (-2cx,-2cy,-2cz, cc, 1) -> bands (0,1,2, 6, 7)
    # bcb: wlo rows (lox,loy,loz, junk) -> bands (3,4,5, -)
    bca = np.zeros((5, NB * J), np.float32)
    bcb = np.zeros((4, NB * J), np.float32)
    for j in range(J):
        for r, bnd in enumerate((0, 1, 2, 6, 7)):
            bca[r, bnd * J + j] = 1.0
        for r, bnd in enumerate((3, 4, 5)):
            bcb[r, bnd * J + j] = 1.0
    ident = np.eye(K, dtype=np.float32)
    return diagk, csel, bca, bcb, ident


_NC_CACHE = {}
TRACE = False
LAST_RESULTS = None


def _get_nc(n_img, iters, groups):
    key = (n_img, iters, groups)
    if key not in _NC_CACHE:
        _NC_CACHE[key] = build_kernel(n_img, iters, groups)
    return _NC_CACHE[key]


def kernel(inputs: np.ndarray) -> np.ndarray:
    x = np.ascontiguousarray(np.asarray(inputs, dtype=np.float32))
    assert x.shape == (B, H, W, D), x.shape
    pixels = x.reshape(B, N, D)

    perm8 = np.array(PERM8, dtype=np.int64)
    cent0 = np.take_along_axis(
        pixels, perm8[:, :, None].repeat(D, axis=2), axis=1
    ).astype(np.float32)

    x5, xpix = host_layouts(pixels)
    diagk, csel, bca, bcb, ident = host_constants()
    nc = _get_nc(IMG_PER_CORE, ITERS, GROUPS)

    in_maps = []
    for c in range(NCORES):
        sl = slice(c * IMG_PER_CORE, (c + 1) * IMG_PER_CORE)
        in_maps.append({
            "x5": np.ascontiguousarray(x5[sl]),
            "xpix": np.ascontiguousarray(xpix[sl]),
            "cent0": np.ascontiguousarray(cent0[sl]),
            "diagk": diagk,
            "csel": csel,
            "bca": bca,
            "bcb": bcb,
            "ident": ident,
        })

    global LAST_RESULTS
    try:
        res = run_bass_kernel_spmd(nc, in_maps, core_ids=list(range(NCORES)),
                                   trace=TRACE)
    except Exception:
        if not TRACE:
            raise
        res = run_bass_kernel_spmd(nc, in_maps, core_ids=list(range(NCORES)))
    LAST_RESULTS = res
    outs = [r["cent_out"].reshape(IMG_PER_CORE, K * D) for r in res.results]
    return np.concatenate(outs, axis=0).astype(np.float32)


if __name__ == "__main__":
    rs = np.random.RandomState(0)
    x = rs.random_sample((B, H, W, D)).astype(np.float32)
    out = kernel(inputs=x)
    print("out shape", out.shape, out.dtype)
    print(out[0])


# revision 25
# speedup vs baseline: 1.3067x; 1.0005x over previous
"""Trainium2 Bass kernel for nn_ColorExtractor (per-image k-means, K=8, 10 iters).

Contract: kernel(**inputs) takes FULL inputs ([64, 512, 512, 3] f32), returns
FULL output ([64, 24] f32), batch sharded over 8 NeuronCores (8 images/core).

v4 design (vs v2):
  pipeline fully symmetric cross-image software pipeline: each image's
          final 16-group super-batch of segment-sum matmuls is CARRIED into
          the other image's batch loop and flushed in 8-matmul chunks
          between score bursts, so PSUM->SBUF copies are never head-of-line
          blocked behind a seg backlog; each image's fold/update/weights
          chain is emitted from a hook inside the other image's loop once
          the carried chunks drain. Measured DVE (the bottleneck engine)
          utilization 96%.
  weights w8 = [-2c | sum c^2] built on ACT (mul + Square-with-accum) and
          the centroid-update scale ops ride ACT's per-partition scale,
          freeing DVE cycles.
  seg     stays fp16 (out-partition-64 matmuls with alternating PE column
          tiles). An fp8 DoubleRow variant (mask fp16 buffer byte-aliased
          to (0.0, 1.5*mask) fp8 pairs, xpix fp8 with a zeroed partner
          subtile, the exact 1.5 scale cancelling in means) is implemented
          behind SEG_FP8 but DISABLED: fp8 xpix quantization excites
          k-means trajectory chaos (measured rel err 1.98e-2 vs the 2e-2
          gate; subsampling experiments show the same chaos at 1e-1 scale)
          and the DoubleRow path also serializes LDWEIGHTS on the PE.

Initial centroids replicate jax.random.permutation(key, N)[:8] via the
precomputed PERM8 table (numpy threefry port, verified bit-exact).
"""

import numpy as np

try:                      # only needed for the (disabled) fp8 seg path
    import ml_dtypes
except ImportError:       # pragma: no cover
    ml_dtypes = None

import concourse.bacc as bacc
import concourse.bass as bass
import concourse.tile as tile
from concourse import mybir
from concourse.bass_utils import run_bass_kernel_spmd

# ----------------------------------------------------------------------------
# problem constants (hardcoded per contract)
B = 64            # total images
NCORES = 8
IMG_PER_CORE = B // NCORES
H = W = 512
N = H * W         # pixels per image: 262144
K = 8             # clusters
ITERS = 10
D = 3

# device tiling
P = 128           # pixels per chunk
J = 16            # chunks per matmul group (block-diag j packing)
NB = 8            # contraction bands: r,g,b, r,g,b(lo), ones, |x|^2
GROUPS = N // (J * P)    # 128 groups per image
GBATCH = 8        # groups per PSUM batch
NFILL = 2         # PSUM fills per DVE super-batch
SDB = NFILL * GBATCH     # groups per DVE super-batch
NSUP = GROUPS // SDB
FD = K * J        # 128: free dim of scores/seg matmuls, cols = (k, j)
CJ = 4 * J        # 64: xpix cols per group, (c in {r,g,b,1}, j)

F32 = mybir.dt.float32
F16 = mybir.dt.float16
F8 = mybir.dt.float8e4

# fp8 DoubleRow segment sums (mask byte-alias trick): measured rel err
# 1.98e-2 vs the 2e-2 gate (fp8 xpix quantization excites trajectory chaos
# in k-means) and the DR matmuls serialize LDWEIGHTS. Keep the fp16 path.
SEG_FP8 = False

# ----------------------------------------------------------------------------
# numpy threefry port (verified bit-exact vs jax 0.8 threefry2x32 impl)
_U32 = np.uint32


def _rotl(x, d):
    d = _U32(d)
    return (x << d) | (x >> _U32(32 - d))


def _threefry2x32(k1, k2, x1, x2):
    with np.errstate(over="ignore"):
        ks0, ks1 = _U32(k1), _U32(k2)
        ks2 = _U32(ks0 ^ ks1 ^ _U32(0x1BD11BDA))
        x = [(x1 + ks0).astype(_U32), (x2 + ks1).astype(_U32)]

        def rounds(rots, ka, kb, inc):
            for r in rots:
                x[0] = (x[0] + x[1]).astype(_U32)
                x[1] = _rotl(x[1], r)
                x[1] = x[0] ^ x[1]
            x[0] = (x[0] + ka).astype(_U32)
            x[1] = (x[1] + kb + _U32(inc)).astype(_U32)

        rounds((13, 15, 26, 6), ks1, ks2, 1)
        rounds((17, 29, 16, 24), ks2, ks0, 2)
        rounds((13, 15, 26, 6), ks0, ks1, 3)
        rounds((17, 29, 16, 24), ks1, ks2, 4)
        rounds((13, 15, 26, 6), ks2, ks0, 5)
    return x[0], x[1]


def _tf_split(key, num):
    i = np.arange(num, dtype=np.uint64)
    b1, b2 = _threefry2x32(key[0], key[1],
                           (i >> np.uint64(32)).astype(_U32), i.astype(_U32))
    return np.stack([b1, b2], axis=1)


def _tf_bits(key, n):
    i = np.arange(n, dtype=np.uint64)
    b1, b2 = _threefry2x32(key[0], key[1],
                           (i >> np.uint64(32)).astype(_U32), i.astype(_U32))
    return b1 ^ b2


def jax_permutation_indices(seed, batch, n):
    """perm[b] = jax.random.permutation(split(key(seed), batch)[b], n)."""
    keys = _tf_split(np.array([0, seed], _U32), batch)
    num_rounds = int(np.ceil(3 * np.log(max(1, n)) / np.log(2**32 - 1)))
    perms = []
    for b in range(batch):
        x = np.arange(n)
        k = keys[b]
        for _ in range(num_rounds):
            ks = _tf_split(k, 2)
            k = ks[0]
            sort_keys = _tf_bits(ks[1], n)
            x = x[np.argsort(sort_keys, kind="stable")]
        perms.append(x[:K])
    return np.stack(perms)  # [batch, K]


# Precomputed jax.random.permutation(split(key(42), 64)[b], N)[:8] indices
# (input-independent; verified against the threefry port above).
PERM8 = (
    (121373, 128858, 64733, 199519, 198377, 234239, 198325, 209106),
    (73520, 236184, 209288, 97370, 64322, 228694, 126128, 72161),
    (143944, 27877, 97040, 2149, 10994, 109181, 179954, 54887),
    (147613, 8773, 54262, 44295, 29289, 11407, 31612, 133442),
    (206432, 166428, 5023, 212109, 16365, 21194, 249053, 195143),
    (13257, 110295, 84080, 119151, 246640, 69532, 130091, 105945),
    (14760, 174397, 198857, 826, 140745, 258776, 214608, 163989),
    (184593, 240934, 160738, 23779, 43199, 47433, 94941, 50416),
    (4386, 21260, 129661, 125128, 50701, 200388, 254109, 44816),
    (203980, 230711, 102351, 31296, 161690, 63692, 194032, 60281),
    (170168, 75997, 12072, 137876, 34146, 48636, 181597, 67859),
    (218987, 48148, 224774, 27163, 85280, 163529, 107708, 238871),
    (152153, 120028, 50368, 168498, 254864, 185234, 259971, 5221),
    (126051, 57270, 7614, 194865, 246341, 83824, 226962, 115962),
    (68603, 18235, 201699, 6558, 217064, 74053, 140307, 29320),
    (212222, 174163, 63891, 131714, 260991, 125525, 109871, 254552),
    (208133, 37817, 108871, 236086, 230829, 224735, 197202, 126789),
    (36220, 183667, 173531, 231574, 63007, 23270, 242256, 172824),
    (226174, 181177, 45094, 10219, 172720, 14537, 122494, 27364),
    (19288, 1130, 162371, 12239, 106820, 190833, 228451, 33845),
    (420, 256427, 250298, 234965, 137965, 33886, 192615, 137263),
    (30426, 206099, 1480, 169907, 122972, 5299, 178194, 116853),
    (38366, 252943, 119579, 233642, 99176, 152381, 1818, 246484),
    (49412, 124354, 252000, 221213, 103625, 2726, 153653, 148581),
    (82319, 1626, 107383, 158105, 81846, 13120, 1198, 193305),
    (44406, 239081, 240884, 84662, 7763, 52627, 182256, 187716),
    (185632, 105456, 212756, 173585, 81328, 74972, 128159, 45046),
    (104599, 7215, 61087, 26573, 59314, 48591, 945, 28553),
    (127710, 94893, 75476, 221733, 184125, 96685, 172243, 242612),
    (42647, 29769, 148111, 39823, 193859, 57502, 144317, 214559),
    (780, 145567, 79710, 226978, 2835, 160638, 8378, 24523),
    (161231, 246284, 44873, 150516, 114149, 68239, 117811, 141424),
    (31461, 110744, 232951, 16033, 179041, 106854, 47200, 63782),
    (255322, 241469, 248608, 95048, 170033, 253394, 261582, 181885),
    (63034, 5, 212309, 79222, 1841, 237107, 261430, 22474),
    (203738, 21095, 211942, 6233, 26825, 175918, 126433, 89713),
    (57893, 173681, 13566, 126980, 140303, 73406, 105028, 86705),
    (15800, 76765, 217596, 184873, 201602, 112166, 76158, 112065),
    (110522, 160113, 18684, 10469, 166599, 145226, 99589, 158310),
    (214726, 131223, 109288, 126812, 105792, 167086, 256918, 18441),
    (164736, 182565, 35066, 89660, 98586, 130539, 202194, 16684),
    (24903, 25959, 122313, 26525, 105627, 87218, 23062, 109362),
    (67552, 140412, 247510, 126439, 184322, 171107, 87397, 165128),
    (211326, 162921, 221946, 131793, 156106, 253917, 2345, 133918),
    (219591, 25610, 154884, 239521, 173390, 39973, 114213, 162088),
    (69694, 51180, 74827, 176121, 132947, 148345, 15083, 196459),
    (229624, 100015, 196100, 105569, 78527, 72176, 225549, 208691),
    (158498, 42753, 240006, 246065, 213196, 49877, 129372, 244273),
    (51001, 229538, 39704, 237637, 58774, 83576, 211231, 135814),
    (173630, 162748, 219633, 240928, 8298, 5311, 113776, 113251),
    (64061, 16436, 138070, 47525, 57016, 229742, 159929, 228539),
    (73108, 34503, 7538, 165920, 68681, 114191, 193009, 48042),
    (2842, 97501, 29489, 248778, 176907, 223147, 54452, 11731),
    (224345, 79068, 183290, 239324, 14912, 169078, 122283, 32914),
    (95340, 11646, 45163, 48387, 78062, 60978, 227735, 162106),
    (258986, 131616, 85766, 51383, 132449, 213013, 150516, 231609),
    (65332, 246689, 206208, 181886, 235636, 139183, 132468, 6602),
    (6778, 179487, 58159, 114248, 26277, 180706, 54969, 240497),
    (15413, 19595, 73952, 219244, 68813, 152629, 243501, 175077),
    (208668, 251169, 186627, 98857, 78225, 13125, 12392, 28954),
    (81754, 93281, 49839, 112579, 166016, 88571, 91558, 20863),
    (108264, 245898, 72992, 168504, 68263, 195879, 27596, 23576),
    (44918, 166098, 212537, 239555, 231283, 94408, 203172, 18701),
    (113563, 111669, 16481, 161974, 22111, 116384, 31096, 252828),
)


# ----------------------------------------------------------------------------
# device kernel builder


def build_kernel(n_img=IMG_PER_CORE, iters=ITERS, groups=GROUPS):
    nc = bacc.Bacc("TRN2", target_bir_lowering=False)

    XPW = 2 * CJ if SEG_FP8 else CJ    # xpix cols per group on device
    xpix_dt = F8 if SEG_FP8 else F16

    x5_d = nc.dram_tensor("x5", [n_img, NB * J, groups * P], F16,
                          kind="ExternalInput")
    xpix_d = nc.dram_tensor("xpix", [n_img, P, groups * XPW], xpix_dt,
                            kind="ExternalInput")
    SEGR_ = CJ if SEG_FP8 else 2 * CJ
    c0_d = nc.dram_tensor("cent0", [n_img, K, D], F32, kind="ExternalInput")
    diagk_d = nc.dram_tensor("diagk", [NB * J, FD], F32, kind="ExternalInput")
    csel_d = nc.dram_tensor("csel", [SEGR_, 4], F32, kind="ExternalInput")
    bca_d = nc.dram_tensor("bca", [5, NB * J], F32, kind="ExternalInput")
    bcb_d = nc.dram_tensor("bcb", [4, NB * J], F32, kind="ExternalInput")
    ident_d = nc.dram_tensor("ident", [K, K], F32, kind="ExternalInput")
    out_d = nc.dram_tensor("cent_out", [n_img, K, D], F32, kind="ExternalOutput")

    with tile.TileContext(nc) as tc:
        with (
            tc.tile_pool(name="singles", bufs=1) as singles,
            tc.tile_pool(name="s16p", bufs=3) as s16pool,
            tc.tile_pool(name="maskp", bufs=3) as maskpool,
            tc.tile_pool(name="mvp", bufs=1) as mvpool,
            tc.tile_pool(name="bigpsum", bufs=2, space="PSUM") as bigpsum,
            tc.tile_pool(name="segpsum", bufs=1, space="PSUM") as segpsum,
            tc.tile_pool(name="smallpsum", bufs=1, space="PSUM") as smallps,
        ):
            # --- constants ---
            diagk = singles.tile([NB * J, FD], F32, tag="diagk")
            nc.sync.dma_start(out=diagk[:], in_=diagk_d[:])
            diagk16 = singles.tile([NB * J, FD], F16, tag="diagk16")
            nc.scalar.copy(diagk16[:], diagk[:])
            csel = singles.tile([SEGR_, 4], F32, tag="csel")
            nc.sync.dma_start(out=csel[:], in_=csel_d[:])
            bca = singles.tile([5, NB * J], F32, tag="bca")
            nc.sync.dma_start(out=bca[:], in_=bca_d[:])
            bcb = singles.tile([4, NB * J], F32, tag="bcb")
            nc.sync.dma_start(out=bcb[:], in_=bcb_d[:])
            ident = singles.tile([K, K], F32, tag="ident")
            nc.sync.dma_start(out=ident[:], in_=ident_d[:])

            # --- persistent state ---
            # x5 lives in a 3-slot ring; xpix in a 4-slot ring. Two images
            # (a pair) are interleaved inside the trip loop so each image's
            # serial fold/update/weight chain hides under the other's batch
            # loop; the next pair prefetches during the current one.
            NX5 = 3
            x5t = [singles.tile([NB * J, groups * P], F16, tag=f"x5_{i}",
                                name=f"x5_{i}")
                   for i in range(NX5)]
            NXP = 3
            xpixt = [singles.tile([P, groups * XPW], xpix_dt, tag=f"xp_{i}",
                                  name=f"xp_{i}")
                     for i in range(NXP)]
            # per-pair-slot state (index 0 = image A, 1 = image B)
            cent = [singles.tile([K, D], F32, tag=f"cent{i}", name=f"cent{i}")
                    for i in range(2)]
            w8 = [singles.tile([K, 5], F32, tag=f"w8_{i}", name=f"w8_{i}")
                  for i in range(2)]
            for t in w8:
                nc.vector.memset(t[:, 4:5], 1.0)  # |x|^2 band weight
            csq = [singles.tile([K, D], F32, tag=f"csq{i}", name=f"csq{i}")
                   for i in range(2)]
            wt5 = [singles.tile([5, K], F32, tag=f"wt5_{i}", name=f"wt5_{i}")
                   for i in range(2)]
            whi16 = [singles.tile([4, K], F16, tag=f"whi{i}", name=f"whi{i}")
                     for i in range(2)]
            wlo = [singles.tile([4, K], F32, tag=f"wlo{i}", name=f"wlo{i}")
                   for i in range(2)]
            wrep = [singles.tile([NB * J, K], F16, tag=f"wrp{i}", name=f"wrp{i}")
                    for i in range(2)]
            wdiag16 = [singles.tile([NB * J, FD], F16, tag=f"wd{i}",
                                    name=f"wd{i}")
                       for i in range(2)]
            SEGR = CJ if SEG_FP8 else 2 * CJ   # fold row count
            prod = [singles.tile([SEGR, FD], F32, tag=f"prod{i}",
                                 name=f"prod{i}")
                    for i in range(2)]
            ext = [singles.tile([SEGR, K], F32, tag=f"ext{i}", name=f"ext{i}")
                   for i in range(2)]
            cntc = [singles.tile([K, 1], F32, tag=f"cnt{i}", name=f"cnt{i}")
                    for i in range(2)]
            recip = [singles.tile([K, 1], F32, tag=f"rcp{i}", name=f"rcp{i}")
                     for i in range(2)]
            pos = [singles.tile([K, 1], F32, tag=f"pos{i}", name=f"pos{i}")
                   for i in range(2)]
            cmean = [singles.tile([K, D], F32, tag=f"cm{i}", name=f"cm{i}")
                     for i in range(2)]
            cdel = [singles.tile([K, D], F32, tag=f"cd{i}", name=f"cd{i}")
                    for i in range(2)]

            NQ = 4

            def dma_image(img):
                x5b, xpb = x5t[img % NX5], xpixt[img % NXP]
                w = groups * P // NQ
                for q in range(NQ):
                    nc.sync.dma_start(
                        out=x5b[:, q * w:(q + 1) * w],
                        in_=x5_d[img][:, q * w:(q + 1) * w])
                w2 = groups * XPW // 2
                for q in range(2):
                    nc.sync.dma_start(
                        out=xpb[:, q * w2:(q + 1) * w2],
                        in_=xpix_d[img][:, q * w2:(q + 1) * w2])

            # persistent PSUM accumulators for the two in-flight images.
            # fp8 DoubleRow dst must sit at partition base 0, so the two
            # group-parity accumulation chains live side by side in the free
            # dim ([64, 2*FD]) instead of stacked on partitions.
            if SEG_FP8:
                segt = [segpsum.tile([CJ, 2 * FD], F32, tag=f"seg{i}",
                                     name=f"seg{i}")
                        for i in range(2)]
                segsum = [singles.tile([CJ, FD], F32, tag=f"ss{i}",
                                       name=f"ss{i}")
                          for i in range(2)]
            else:
                segt = [segpsum.tile([2 * CJ, FD], F32, tag=f"seg{i}",
                                     name=f"seg{i}")
                        for i in range(2)]

            def weights_part(sl):
                # ---- weights from centroids ----
                # w8 = [-2c | sum(c^2) | 1]; built on ACT to spare the DVE
                nc.scalar.mul(w8[sl][:, 0:D], cent[sl][:], -2.0)
                nc.scalar.activation(
                    csq[sl][:], cent[sl][:],
                    mybir.ActivationFunctionType.Square,
                    accum_out=w8[sl][:, 3:4])
                wtP = smallps.tile([5, K], F32, tag=f"small{sl}")
                nc.tensor.transpose(wtP[:], w8[sl][:], ident[:])
                nc.scalar.copy(wt5[sl][:], wtP[:])
                # lo-correction rows: wlo = wt - fp16(wt) for the -2c rows
                nc.scalar.copy(whi16[sl][:], wt5[sl][0:4, :])
                nc.vector.tensor_sub(wlo[sl][:], wt5[sl][0:4, :], whi16[sl][:])
                # wrep[(b,j), k] = per-band weight: bands 0-2 <- -2c,
                # 3-5 <- lo(-2c), 6 <- |c|^2, 7 <- 1 (two accumulated MMs
                # with constant selector matrices; no partition shifts)
                wrepP = smallps.tile([NB * J, K], F32, tag=f"small{sl}")
                nc.tensor.matmul(wrepP[:], bca[:], wt5[sl][:],
                                 start=True, stop=False)
                nc.tensor.matmul(wrepP[:], bcb[:], wlo[sl][:],
                                 start=False, stop=True)
                nc.scalar.copy(wrep[sl][:], wrepP[:])
                # wdiag16[(b,j), (k,j')] = wrep[(b,j), k] * 1[j==j']
                # (all-fp16 operands keep the DVE in 2x packed mode; the
                # fp16 rounding of wrep matches wdiag16's own rounding)
                wrep_b = bass.AP(
                    tensor=wrep[sl][:].tensor, offset=wrep[sl][:].offset,
                    ap=[wrep[sl][:].ap[0], [1, K], [0, J]])
                nc.vector.tensor_tensor(
                    wdiag16[sl][:].rearrange("p (k j) -> p k j", j=J),
                    diagk16[:].rearrange("p (k j) -> p k j", j=J),
                    wrep_b, mybir.AluOpType.mult)

            def batches_part(sl, x5b, xpb, carry=None, hook=None):
                # ---- main loop over super-batches. Seg matmuls trail their
                # mask by one super-batch, flushed in 8-MM chunks between
                # score bursts so the PE interleaves scores and seg work and
                # copies are never head-of-line blocked. The final super-
                # batch's chunks are RETURNED and flush inside the OTHER
                # image's batch loop (cross-image carry); `hook` emits that
                # image's fold/weights chain once the carry has drained. ----
                seg = segt[sl]
                flushq = list(carry) if carry else []

                if SEG_FP8:
                    def seg_chunk(mk, q, c):
                        def go():
                            mk8 = mk[:].bitcast(F8).rearrange(
                                "p (n two) -> p two n", two=2)
                            for t in range(c * 8, c * 8 + 8):
                                g = q * SDB + t
                                hf = g & 1
                                xp8 = xpb[:, g * XPW:(g + 1) * XPW].rearrange(
                                    "p (two c) -> p two c", two=2)
                                nc.tensor.matmul(
                                    seg[:, hf * FD:(hf + 1) * FD],
                                    xp8,
                                    mk8[:, :, t * FD:(t + 1) * FD],
                                    perf_mode=mybir.MatmulPerfMode.DoubleRow,
                                    start=(g == hf),
                                    stop=(g == groups - 2 + hf),
                                    skip_group_check=True)
                        return go
                else:
                    def seg_chunk(mk, q, c):
                        def go():
                            for t in range(c * 8, c * 8 + 8):
                                g = q * SDB + t
                                hf = g & 1
                                nc.tensor.matmul(
                                    seg[hf * CJ:(hf + 1) * CJ, :],
                                    xpb[:, g * CJ:(g + 1) * CJ],
                                    mk[:, t * FD:(t + 1) * FD],
                                    start=(g == hf),
                                    stop=(g == groups - 2 + hf),
                                    skip_group_check=True,
                                    tile_position=(0, hf * CJ))
                        return go

                for q in range(NSUP):
                    s16 = s16pool.tile([P, SDB * FD], F16, tag="s16")
                    if q == 1 and hook is not None:
                        hook()
                    for h in range(NFILL):
                        sp = bigpsum.tile([P, GBATCH * FD], F32, tag="big")
                        for t in range(GBATCH):
                            g = (q * NFILL + h) * GBATCH + t
                            nc.tensor.matmul(
                                sp[:, t * FD:(t + 1) * FD],
                                x5b[:, g * P:(g + 1) * P],
                                wdiag16[sl][:], start=True, stop=True)
                        if flushq:
                            flushq.pop(0)()
                        # ACT evacuates scores to fp16 SBUF (k-outer layout)
                        nc.scalar.copy(
                            s16[:, h * GBATCH * FD:(h + 1) * GBATCH * FD],
                            sp[:])
                    s4 = s16[:].rearrange("p (gb k j) -> p gb k j", k=K, j=J)
                    # 3-round pairwise min over k (all 2x packed-16 mode)
                    mv1 = mvpool.tile([P, SDB * 4 * J], F16, tag="mv1")
                    m1 = mv1[:].rearrange("p (gb k j) -> p gb k j", k=4, j=J)
                    nc.vector.tensor_tensor(
                        m1, s4[:, :, 0:4, :], s4[:, :, 4:8, :],
                        mybir.AluOpType.min)
                    mv2 = mvpool.tile([P, SDB * 2 * J], F16, tag="mv2")
                    m2 = mv2[:].rearrange("p (gb k j) -> p gb k j", k=2, j=J)
                    nc.vector.tensor_tensor(
                        m2, m1[:, :, 0:2, :], m1[:, :, 2:4, :],
                        mybir.AluOpType.min)
                    mv3 = mvpool.tile([P, SDB * J], F16, tag="mv3")
                    m3 = mv3[:].rearrange("p (gb j) -> p gb j", j=J)
                    nc.vector.tensor_tensor(
                        m3.unsqueeze(2), m2[:, :, 0:1, :], m2[:, :, 1:2, :],
                        mybir.AluOpType.min)
                    # mask = (s16 <= min) broadcast over k
                    mv_b = bass.AP(
                        tensor=mv3[:].tensor, offset=mv3[:].offset,
                        ap=[mv3[:].ap[0], [J, SDB], [0, K], [1, J]])
                    mk = maskpool.tile([P, SDB * FD], F16, tag="mk")
                    nc.vector.tensor_tensor(
                        mk[:].rearrange("p (gb k j) -> p gb k j", k=K, j=J),
                        s4, mv_b, mybir.AluOpType.is_le)
                    flushq.extend(seg_chunk(mk, q, c) for c in range(SDB // 8))
                return flushq

            def fold_update_part(sl):
                # ---- fold seg -> S[k, (r,g,b,count)] ----
                if SEG_FP8:
                    # sum the two parity chains, then one 64-row fold
                    # (only one PSUM operand allowed per instruction)
                    nc.scalar.copy(segsum[sl][:], segt[sl][:, 0:FD])
                    nc.vector.tensor_add(
                        segsum[sl][:], segsum[sl][:], segt[sl][:, FD:])
                    nc.vector.tensor_tensor(
                        prod[sl][:], segsum[sl][:], diagk[0:CJ, :],
                        mybir.AluOpType.mult)
                else:
                    nc.vector.tensor_tensor(
                        prod[sl][:], segt[sl][:], diagk[:],
                        mybir.AluOpType.mult)
                nc.vector.tensor_reduce(
                    ext[sl][:],
                    prod[sl][:].rearrange("p (k j) -> p k j", j=J),
                    axis=mybir.AxisListType.X,
                    op=mybir.AluOpType.add)
                S = smallps.tile([K, 4], F32, tag=f"small{sl}")
                nc.tensor.matmul(S[:], ext[sl][:], csel[:],
                                 start=True, stop=True)

                # ---- centroid update ----
                # counts come back scaled by 1.5 in fp8 mode; threshold 0.5
                # keeps n=0 -> mean 0 (masked) and n>=1 exact in both modes.
                nc.vector.tensor_scalar_max(cntc[sl][:], S[:, 3:4], 0.5)
                nc.vector.reciprocal(recip[sl][:], cntc[sl][:])
                # per-partition scale rides the ACT ops (spares the DVE)
                nc.scalar.mul(cmean[sl][:], S[:, 0:D], recip[sl][:])
                nc.vector.tensor_scalar(
                    pos[sl][:], S[:, 3:4], 0.5, None,
                    op0=mybir.AluOpType.is_ge)
                nc.vector.tensor_sub(cdel[sl][:], cmean[sl][:], cent[sl][:])
                nc.scalar.mul(cdel[sl][:], cdel[sl][:], pos[sl][:])
                nc.vector.tensor_add(cent[sl][:], cent[sl][:], cdel[sl][:])

            nc.sync.dma_start(out=cent[0][:], in_=c0_d[0])
            nc.sync.dma_start(out=cent[1][:], in_=c0_d[1])
            dma_image(0)
            dma_image(1)
            weights_part(0)              # pair 0's A weights

            for pair in range(n_img // 2):
                a, b = 2 * pair, 2 * pair + 1
                # prefetch next pair's A image; its x5/xpix slots are
                # unused by this pair. B's slot aliases image a's, so its
                # prefetch is issued after the trip loop below.
                if a + 2 < n_img:
                    dma_image(a + 2)

                xa, pa = x5t[a % NX5], xpixt[a % NXP]
                xb, pb = x5t[b % NX5], xpixt[b % NXP]

                # Fully symmetric software pipeline: each image's
                # fold/update/weights chain is emitted from a hook INSIDE
                # the other image's batch loop (after the carried seg
                # chunks drain), and each image's final super-batch of seg
                # matmuls flushes interleaved with the other image's score
                # bursts. On the last trip, A's result ships early and the
                # NEXT pair's A-centroids + weight chain are built under
                # B's final batch loop.
                def hook_b0():
                    weights_part(1)

                def hook_a():
                    fold_update_part(1)
                    weights_part(1)

                def hook_b():
                    fold_update_part(0)
                    weights_part(0)

                def hook_b_last():
                    fold_update_part(0)
                    nc.sync.dma_start(out=out_d[a], in_=cent[0][:])
                    if a + 2 < n_img:
                        nc.sync.dma_start(out=cent[0][:], in_=c0_d[a + 2])
                        weights_part(0)   # next pair's A weights

                carry = batches_part(0, xa, pa, hook=hook_b0)
                carry = batches_part(1, xb, pb, carry=carry, hook=hook_b)
                for t in range(1, iters):
                    carry = batches_part(0, xa, pa, carry=carry, hook=hook_a)
                    carry = batches_part(
                        1, xb, pb, carry=carry,
                        hook=hook_b if t < iters - 1 else hook_b_last)
                for chunk in carry:
                    chunk()
                fold_update_part(1)

                nc.sync.dma_start(out=out_d[b], in_=cent[1][:])
                if b + 2 < n_img:
                    nc.sync.dma_start(out=cent[1][:], in_=c0_d[b + 2])
                    dma_image(b + 2)

    nc.finalize()
    return nc


# ----------------------------------------------------------------------------
# host-side layouts


def host_layouts(pixels):
    """pixels [B, N, 3] f32 -> (x5 [B, 128, 16384] f16, xpix fp8/fp16).

    x5[(b,j), (g,p)]: bands 0-2 / 3-5 = x_rgb fp16 (hi/lo share data),
    band 6 = 1.0, band 7 = |x|^2 fp16, for pixel g*J*P + j*P + p.
    xpix fp8 mode: [p, (g, s, c, j)], s=0 zeros (pairs with the always-zero
    low mask byte), s=1 = fp8 pixel values; c in {r,g,b,1}.
    """
    b = pixels.shape[0]
    g = GROUPS
    v = pixels.reshape(b, g, J, P, D)
    rgb = np.ascontiguousarray(
        v.transpose(0, 4, 2, 1, 3).reshape(b, D * J, g * P)).astype(np.float16)
    xsq = (pixels.astype(np.float32) ** 2).sum(-1).astype(np.float16)
    xsqr = np.ascontiguousarray(
        xsq.reshape(b, g, J, P).transpose(0, 2, 1, 3).reshape(b, J, g * P))
    x5 = np.empty((b, NB * J, g * P), np.float16)
    x5[:, 0:48] = rgb
    x5[:, 48:96] = rgb
    x5[:, 96:112] = np.float16(1.0)
    x5[:, 112:128] = xsqr
    if SEG_FP8:
        xp = np.zeros((b, P, g, 2, 4, J), ml_dtypes.float8_e4m3)
        xp[..., 1, 0:3, :] = v.transpose(0, 3, 1, 4, 2).astype(
            ml_dtypes.float8_e4m3)  # b p g c j
        xp[..., 1, 3, :] = ml_dtypes.float8_e4m3(1.0)
        xpix = np.ascontiguousarray(xp.reshape(b, P, g * 2 * CJ))
    else:
        xp = np.empty((b, P, g, 4, J), np.float16)
        xp[..., 0:3, :] = v.transpose(0, 3, 1, 4, 2).astype(np.float16)
        xp[..., 3, :] = np.float16(1.0)
        xpix = np.ascontiguousarray(xp.reshape(b, P, g * CJ))
    return x5, xpix


def host_constants():
    diagk = np.zeros((NB * J, FD), np.float32)
    for bnd in range(NB):
        for j in range(J):
            for k in range(K):
                diagk[bnd * J + j, k * J + j] = 1.0
    if SEG_FP8:
        csel = np.zeros((CJ, 4), np.float32)
        for c in range(4):
            for j in range(J):
                csel[c * J + j, c] = 1.0
    else:
        csel = np.zeros((2 * CJ, 4), np.float32)
        for h in range(2):
            for c in range(4):
                for j in range(J):
                    csel[h * CJ + c * J + j, c] = 1.0
    # bca: wt5 rows (-2cx,-2cy,-2cz, cc, 1) -> bands (0,1,2, 6, 7)
    # bcb: wlo rows (lox,loy,loz, junk) -> bands (3,4,5, -)
    bca = np.zeros((5, NB * J), np.float32)
    bcb = np.zeros((4, NB * J), np.float32)
    for j in range(J):
        for r, bnd in enumerate((0, 1, 2, 6, 7)):
            bca[r, bnd * J + j] = 1.0
        for r, bnd in enumerate((3, 4, 5)):
            bcb[r, bnd * J + j] = 1.0
    ident = np.eye(K, dtype=np.float32)
    return diagk, csel, bca, bcb, ident


_NC_CACHE = {}
TRACE = False
LAST_RESULTS = None


def _get_nc(n_img, iters, groups):
    key = (n_img, iters, groups)
    if key not in _NC_CACHE:
        _NC_CACHE[key] = build_kernel(n_img, iters, groups)
    return _NC_CACHE[key]


def kernel(inputs: np.ndarray) -> np.ndarray:
    x = np.ascontiguousarray(np.asarray(inputs, dtype=np.float32))
    assert x.shape == (B, H, W, D), x.shape
    pixels = x.reshape(B, N, D)

    perm8 = np.array(PERM8, dtype=np.int64)
    cent0 = np.take_along_axis(
        pixels, perm8[:, :, None].repeat(D, axis=2), axis=1
    ).astype(np.float32)

    x5, xpix = host_layouts(pixels)
    diagk, csel, bca, bcb, ident = host_constants()
    nc = _get_nc(IMG_PER_CORE, ITERS, GROUPS)

    in_maps = []
    for c in range(NCORES):
        sl = slice(c * IMG_PER_CORE, (c + 1) * IMG_PER_CORE)
        in_maps.append({
            "x5": np.ascontiguousarray(x5[sl]),
            "xpix": np.ascontiguousarray(xpix[sl]),
            "cent0": np.ascontiguousarray(cent0[sl]),
            "diagk": diagk,
            "csel": csel,
            "bca": bca,
            "bcb": bcb,
            "ident": ident,
        })

    global LAST_RESULTS
    try:
        res = run_bass_kernel_spmd(nc, in_maps, core_ids=list(range(NCORES)),
                                   trace=TRACE)
    except Exception:
        if not TRACE:
            raise
        res = run_bass_kernel_spmd(nc, in_maps, core_ids=list(range(NCORES)))
    LAST_RESULTS = res
    outs = [r["cent_out"].reshape(IMG_PER_CORE, K * D) for r in res.results]
    return np.concatenate(outs, axis=0).astype(np.float32)


if __name__ == "__main__":
    rs = np.random.RandomState(0)
    x = rs.random_sample((B, H, W, D)).astype(np.float32)
    out = kernel(inputs=x)
    print("out shape", out.shape, out.dtype)
    print(out[0])


# revision 28
# speedup vs baseline: 1.3102x; 1.0027x over previous
"""Trainium2 Bass kernel for nn_ColorExtractor (per-image k-means, K=8, 10 iters).

Contract: kernel(**inputs) takes FULL inputs ([64, 512, 512, 3] f32), returns
FULL output ([64, 24] f32), batch sharded over 8 NeuronCores (8 images/core).

v4 design (vs v2):
  pipeline fully symmetric cross-image software pipeline: each image's
          final 16-group super-batch of segment-sum matmuls is CARRIED into
          the other image's batch loop and flushed in 8-matmul chunks
          between score bursts, so PSUM->SBUF copies are never head-of-line
          blocked behind a seg backlog; each image's fold/update/weights
          chain is emitted from a hook inside the other image's loop once
          the carried chunks drain. Measured DVE (the bottleneck engine)
          utilization 96%.
  weights w8 = [-2c | sum c^2] built on ACT (mul + Square-with-accum) and
          the centroid-update scale ops ride ACT's per-partition scale,
          freeing DVE cycles.
  seg     stays fp16 (out-partition-64 matmuls with alternating PE column
          tiles). An fp8 DoubleRow variant (mask fp16 buffer byte-aliased
          to (0.0, 1.5*mask) fp8 pairs, xpix fp8 with a zeroed partner
          subtile, the exact 1.5 scale cancelling in means) is implemented
          behind SEG_FP8 but DISABLED: fp8 xpix quantization excites
          k-means trajectory chaos (measured rel err 1.98e-2 vs the 2e-2
          gate; subsampling experiments show the same chaos at 1e-1 scale)
          and the DoubleRow path also serializes LDWEIGHTS on the PE.

Initial centroids replicate jax.random.permutation(key, N)[:8] via the
precomputed PERM8 table (numpy threefry port, verified bit-exact).
"""

import numpy as np

try:                      # only needed for the (disabled) fp8 seg path
    import ml_dtypes
except ImportError:       # pragma: no cover
    ml_dtypes = None

import concourse.bacc as bacc
import concourse.bass as bass
import concourse.tile as tile
from concourse import mybir
from concourse.bass_utils import run_bass_kernel_spmd

# ----------------------------------------------------------------------------
# problem constants (hardcoded per contract)
B = 64            # total images
NCORES = 8
IMG_PER_CORE = B // NCORES
H = W = 512
N = H * W         # pixels per image: 262144
K = 8             # clusters
ITERS = 10
D = 3

# device tiling
P = 128           # pixels per chunk
J = 16            # chunks per matmul group (block-diag j packing)
NB = 8            # contraction bands: r,g,b, r,g,b(lo), ones, |x|^2
GROUPS = N // (J * P)    # 128 groups per image
GBATCH = 8        # groups per PSUM batch
NFILL = 2         # PSUM fills per DVE super-batch
SDB = NFILL * GBATCH     # groups per DVE super-batch
NSUP = GROUPS // SDB
FD = K * J        # 128: free dim of scores/seg matmuls, cols = (k, j)
CJ = 4 * J        # 64: xpix cols per group, (c in {r,g,b,1}, j)

F32 = mybir.dt.float32
F16 = mybir.dt.float16
F8 = mybir.dt.float8e4

# fp8 DoubleRow segment sums (mask byte-alias trick): measured rel err
# 1.98e-2 vs the 2e-2 gate (fp8 xpix quantization excites trajectory chaos
# in k-means) and the DR matmuls serialize LDWEIGHTS. Keep the fp16 path.
SEG_FP8 = False

# ----------------------------------------------------------------------------
# numpy threefry port (verified bit-exact vs jax 0.8 threefry2x32 impl)
_U32 = np.uint32


def _rotl(x, d):
    d = _U32(d)
    return (x << d) | (x >> _U32(32 - d))


def _threefry2x32(k1, k2, x1, x2):
    with np.errstate(over="ignore"):
        ks0, ks1 = _U32(k1), _U32(k2)
        ks2 = _U32(ks0 ^ ks1 ^ _U32(0x1BD11BDA))
        x = [(x1 + ks0).astype(_U32), (x2 + ks1).astype(_U32)]

        def rounds(rots, ka, kb, inc):
            for r in rots:
                x[0] = (x[0] + x[1]).astype(_U32)
                x[1] = _rotl(x[1], r)
                x[1] = x[0] ^ x[1]
            x[0] = (x[0] + ka).astype(_U32)
            x[1] = (x[1] + kb + _U32(inc)).astype(_U32)

        rounds((13, 15, 26, 6), ks1, ks2, 1)
        rounds((17, 29, 16, 24), ks2, ks0, 2)
        rounds((13, 15, 26, 6), ks0, ks1, 3)
        rounds((17, 29, 16, 24), ks1, ks2, 4)
        rounds((13, 15, 26, 6), ks2, ks0, 5)
    return x[0], x[1]


def _tf_split(key, num):
    i = np.arange(num, dtype=np.uint64)
    b1, b2 = _threefry2x32(key[0], key[1],
                           (i >> np.uint64(32)).astype(_U32), i.astype(_U32))
    return np.stack([b1, b2], axis=1)


def _tf_bits(key, n):
    i = np.arange(n, dtype=np.uint64)
    b1, b2 = _threefry2x32(key[0], key[1],
                           (i >> np.uint64(32)).astype(_U32), i.astype(_U32))
    return b1 ^ b2


def jax_permutation_indices(seed, batch, n):
    """perm[b] = jax.random.permutation(split(key(seed), batch)[b], n)."""
    keys = _tf_split(np.array([0, seed], _U32), batch)
    num_rounds = int(np.ceil(3 * np.log(max(1, n)) / np.log(2**32 - 1)))
    perms = []
    for b in range(batch):
        x = np.arange(n)
        k = keys[b]
        for _ in range(num_rounds):
            ks = _tf_split(k, 2)
            k = ks[0]
            sort_keys = _tf_bits(ks[1], n)
            x = x[np.argsort(sort_keys, kind="stable")]
        perms.append(x[:K])
    return np.stack(perms)  # [batch, K]


# Precomputed jax.random.permutation(split(key(42), 64)[b], N)[:8] indices
# (input-independent; verified against the threefry port above).
PERM8 = (
    (121373, 128858, 64733, 199519, 198377, 234239, 198325, 209106),
    (73520, 236184, 209288, 97370, 64322, 228694, 126128, 72161),
    (143944, 27877, 97040, 2149, 10994, 109181, 179954, 54887),
    (147613, 8773, 54262, 44295, 29289, 11407, 31612, 133442),
    (206432, 166428, 5023, 212109, 16365, 21194, 249053, 195143),
    (13257, 110295, 84080, 119151, 246640, 69532, 130091, 105945),
    (14760, 174397, 198857, 826, 140745, 258776, 214608, 163989),
    (184593, 240934, 160738, 23779, 43199, 47433, 94941, 50416),
    (4386, 21260, 129661, 125128, 50701, 200388, 254109, 44816),
    (203980, 230711, 102351, 31296, 161690, 63692, 194032, 60281),
    (170168, 75997, 12072, 137876, 34146, 48636, 181597, 67859),
    (218987, 48148, 224774, 27163, 85280, 163529, 107708, 238871),
    (152153, 120028, 50368, 168498, 254864, 185234, 259971, 5221),
    (126051, 57270, 7614, 194865, 246341, 83824, 226962, 115962),
    (68603, 18235, 201699, 6558, 217064, 74053, 140307, 29320),
    (212222, 174163, 63891, 131714, 260991, 125525, 109871, 254552),
    (208133, 37817, 108871, 236086, 230829, 224735, 197202, 126789),
    (36220, 183667, 173531, 231574, 63007, 23270, 242256, 172824),
    (226174, 181177, 45094, 10219, 172720, 14537, 122494, 27364),
    (19288, 1130, 162371, 12239, 106820, 190833, 228451, 33845),
    (420, 256427, 250298, 234965, 137965, 33886, 192615, 137263),
    (30426, 206099, 1480, 169907, 122972, 5299, 178194, 116853),
    (38366, 252943, 119579, 233642, 99176, 152381, 1818, 246484),
    (49412, 124354, 252000, 221213, 103625, 2726, 153653, 148581),
    (82319, 1626, 107383, 158105, 81846, 13120, 1198, 193305),
    (44406, 239081, 240884, 84662, 7763, 52627, 182256, 187716),
    (185632, 105456, 212756, 173585, 81328, 74972, 128159, 45046),
    (104599, 7215, 61087, 26573, 59314, 48591, 945, 28553),
    (127710, 94893, 75476, 221733, 184125, 96685, 172243, 242612),
    (42647, 29769, 148111, 39823, 193859, 57502, 144317, 214559),
    (780, 145567, 79710, 226978, 2835, 160638, 8378, 24523),
    (161231, 246284, 44873, 150516, 114149, 68239, 117811, 141424),
    (31461, 110744, 232951, 16033, 179041, 106854, 47200, 63782),
    (255322, 241469, 248608, 95048, 170033, 253394, 261582, 181885),
    (63034, 5, 212309, 79222, 1841, 237107, 261430, 22474),
    (203738, 21095, 211942, 6233, 26825, 175918, 126433, 89713),
    (57893, 173681, 13566, 126980, 140303, 73406, 105028, 86705),
    (15800, 76765, 217596, 184873, 201602, 112166, 76158, 112065),
    (110522, 160113, 18684, 10469, 166599, 145226, 99589, 158310),
    (214726, 131223, 109288, 126812, 105792, 167086, 256918, 18441),
    (164736, 182565, 35066, 89660, 98586, 130539, 202194, 16684),
    (24903, 25959, 122313, 26525, 105627, 87218, 23062, 109362),
    (67552, 140412, 247510, 126439, 184322, 171107, 87397, 165128),
    (211326, 162921, 221946, 131793, 156106, 253917, 2345, 133918),
    (219591, 25610, 154884, 239521, 173390, 39973, 114213, 162088),
    (69694, 51180, 74827, 176121, 132947, 148345, 15083, 196459),
    (229624, 100015, 196100, 105569, 78527, 72176, 225549, 208691),
    (158498, 42753, 240006, 246065, 213196, 49877, 129372, 244273),
    (51001, 229538, 39704, 237637, 58774, 83576, 211231, 135814),
    (173630, 162748, 219633, 240928, 8298, 5311, 113776, 113251),
    (64061, 16436, 138070, 47525, 57016, 229742, 159929, 228539),
    (73108, 34503, 7538, 165920, 68681, 114191, 193009, 48042),
    (2842, 97501, 29489, 248778, 176907, 223147, 54452, 11731),
    (224345, 79068, 183290, 239324, 14912, 169078, 122283, 32914),
    (95340, 11646, 45163, 48387, 78062, 60978, 227735, 162106),
    (258986, 131616, 85766, 51383, 132449, 213013, 150516, 231609),
    (65332, 246689, 206208, 181886, 235636, 139183, 132468, 6602),
    (6778, 179487, 58159, 114248, 26277, 180706, 54969, 240497),
    (15413, 19595, 73952, 219244, 68813, 152629, 243501, 175077),
    (208668, 251169, 186627, 98857, 78225, 13125, 12392, 28954),
    (81754, 93281, 49839, 112579, 166016, 88571, 91558, 20863),
    (108264, 245898, 72992, 168504, 68263, 195879, 27596, 23576),
    (44918, 166098, 212537, 239555, 231283, 94408, 203172, 18701),
    (113563, 111669, 16481, 161974, 22111, 116384, 31096, 252828),
)


# ----------------------------------------------------------------------------
# device kernel builder


def build_kernel(n_img=IMG_PER_CORE, iters=ITERS, groups=GROUPS):
    nc = bacc.Bacc("TRN2", target_bir_lowering=False)

    XPW = 2 * CJ if SEG_FP8 else CJ    # xpix cols per group on device
    xpix_dt = F8 if SEG_FP8 else F16

    x5_d = nc.dram_tensor("x5", [n_img, NB * J, groups * P], F16,
                          kind="ExternalInput")
    xpix_d = nc.dram_tensor("xpix", [n_img, P, groups * XPW], xpix_dt,
                            kind="ExternalInput")
    SEGR_ = CJ if SEG_FP8 else 2 * CJ
    c0_d = nc.dram_tensor("cent0", [n_img, K, D], F32, kind="ExternalInput")
    diagk_d = nc.dram_tensor("diagk", [NB * J, FD], F32, kind="ExternalInput")
    csel_d = nc.dram_tensor("csel", [SEGR_, 4], F32, kind="ExternalInput")
    bca_d = nc.dram_tensor("bca", [5, NB * J], F32, kind="ExternalInput")
    bcb_d = nc.dram_tensor("bcb", [4, NB * J], F32, kind="ExternalInput")
    ident_d = nc.dram_tensor("ident", [K, K], F32, kind="ExternalInput")
    out_d = nc.dram_tensor("cent_out", [n_img, K, D], F32, kind="ExternalOutput")

    with tile.TileContext(nc) as tc:
        with (
            tc.tile_pool(name="singles", bufs=1) as singles,
            tc.tile_pool(name="s16p", bufs=3) as s16pool,
            tc.tile_pool(name="maskp", bufs=3) as maskpool,
            tc.tile_pool(name="mvp", bufs=1) as mvpool,
            tc.tile_pool(name="bigpsum", bufs=2, space="PSUM") as bigpsum,
            tc.tile_pool(name="segpsum", bufs=1, space="PSUM") as segpsum,
            tc.tile_pool(name="smallpsum", bufs=1, space="PSUM") as smallps,
        ):
            # --- constants ---
            diagk = singles.tile([NB * J, FD], F32, tag="diagk")
            nc.sync.dma_start(out=diagk[:], in_=diagk_d[:])
            diagk16 = singles.tile([NB * J, FD], F16, tag="diagk16")
            nc.scalar.copy(diagk16[:], diagk[:])
            csel = singles.tile([SEGR_, 4], F32, tag="csel")
            nc.sync.dma_start(out=csel[:], in_=csel_d[:])
            bca = singles.tile([5, NB * J], F32, tag="bca")
            nc.sync.dma_start(out=bca[:], in_=bca_d[:])
            bcb = singles.tile([4, NB * J], F32, tag="bcb")
            nc.sync.dma_start(out=bcb[:], in_=bcb_d[:])
            ident = singles.tile([K, K], F32, tag="ident")
            nc.sync.dma_start(out=ident[:], in_=ident_d[:])

            # --- persistent state ---
            # x5 lives in a 3-slot ring; xpix in a 4-slot ring. Two images
            # (a pair) are interleaved inside the trip loop so each image's
            # serial fold/update/weight chain hides under the other's batch
            # loop; the next pair prefetches during the current one.
            NX5 = 3
            x5t = [singles.tile([NB * J, groups * P], F16, tag=f"x5_{i}",
                                name=f"x5_{i}")
                   for i in range(NX5)]
            NXP = 3
            xpixt = [singles.tile([P, groups * XPW], xpix_dt, tag=f"xp_{i}",
                                  name=f"xp_{i}")
                     for i in range(NXP)]
            # per-pair-slot state (index 0 = image A, 1 = image B)
            cent = [singles.tile([K, D], F32, tag=f"cent{i}", name=f"cent{i}")
                    for i in range(2)]
            w8 = [singles.tile([K, 5], F32, tag=f"w8_{i}", name=f"w8_{i}")
                  for i in range(2)]
            for t in w8:
                nc.vector.memset(t[:, 4:5], 1.0)  # |x|^2 band weight
            csq = [singles.tile([K, D], F32, tag=f"csq{i}", name=f"csq{i}")
                   for i in range(2)]
            wt5 = [singles.tile([5, K], F32, tag=f"wt5_{i}", name=f"wt5_{i}")
                   for i in range(2)]
            whi16 = [singles.tile([4, K], F16, tag=f"whi{i}", name=f"whi{i}")
                     for i in range(2)]
            wlo = [singles.tile([4, K], F32, tag=f"wlo{i}", name=f"wlo{i}")
                   for i in range(2)]
            wrep = [singles.tile([NB * J, K], F16, tag=f"wrp{i}", name=f"wrp{i}")
                    for i in range(2)]
            wdiag16 = [singles.tile([NB * J, FD], F16, tag=f"wd{i}",
                                    name=f"wd{i}")
                       for i in range(2)]
            segsb = [singles.tile([2 * CJ, FD], F32, tag=f"sgb{i}",
                                  name=f"sgb{i}")
                     for i in range(2)]
            SEGR = CJ if SEG_FP8 else 2 * CJ   # fold row count
            prod = [singles.tile([SEGR, FD], F32, tag=f"prod{i}",
                                 name=f"prod{i}")
                    for i in range(2)]
            ext = [singles.tile([SEGR, K], F32, tag=f"ext{i}", name=f"ext{i}")
                   for i in range(2)]
            cntc = [singles.tile([K, 1], F32, tag=f"cnt{i}", name=f"cnt{i}")
                    for i in range(2)]
            recip = [singles.tile([K, 1], F32, tag=f"rcp{i}", name=f"rcp{i}")
                     for i in range(2)]
            pos = [singles.tile([K, 1], F32, tag=f"pos{i}", name=f"pos{i}")
                   for i in range(2)]
            cmean = [singles.tile([K, D], F32, tag=f"cm{i}", name=f"cm{i}")
                     for i in range(2)]
            cdel = [singles.tile([K, D], F32, tag=f"cd{i}", name=f"cd{i}")
                    for i in range(2)]

            NQ = 4

            def dma_image(img):
                x5b, xpb = x5t[img % NX5], xpixt[img % NXP]
                w = groups * P // NQ
                for q in range(NQ):
                    nc.sync.dma_start(
                        out=x5b[:, q * w:(q + 1) * w],
                        in_=x5_d[img][:, q * w:(q + 1) * w])
                w2 = groups * XPW // 2
                for q in range(2):
                    nc.sync.dma_start(
                        out=xpb[:, q * w2:(q + 1) * w2],
                        in_=xpix_d[img][:, q * w2:(q + 1) * w2])

            # persistent PSUM accumulators for the two in-flight images.
            # fp8 DoubleRow dst must sit at partition base 0, so the two
            # group-parity accumulation chains live side by side in the free
            # dim ([64, 2*FD]) instead of stacked on partitions.
            if SEG_FP8:
                segt = [segpsum.tile([CJ, 2 * FD], F32, tag=f"seg{i}",
                                     name=f"seg{i}")
                        for i in range(2)]
                segsum = [singles.tile([CJ, FD], F32, tag=f"ss{i}",
                                       name=f"ss{i}")
                          for i in range(2)]
            else:
                segt = [segpsum.tile([2 * CJ, FD], F32, tag=f"seg{i}",
                                     name=f"seg{i}")
                        for i in range(2)]

            def weights_part(sl):
                # ---- weights from centroids ----
                # w8 = [-2c | sum(c^2) | 1]; built on ACT to spare the DVE
                nc.scalar.mul(w8[sl][:, 0:D], cent[sl][:], -2.0)
                nc.scalar.activation(
                    csq[sl][:], cent[sl][:],
                    mybir.ActivationFunctionType.Square,
                    accum_out=w8[sl][:, 3:4])
                wtP = smallps.tile([5, K], F32, tag=f"small{sl}")
                nc.tensor.transpose(wtP[:], w8[sl][:], ident[:])
                nc.scalar.copy(wt5[sl][:], wtP[:])
                # lo-correction rows: wlo = wt - fp16(wt) for the -2c rows
                nc.scalar.copy(whi16[sl][:], wt5[sl][0:4, :])
                nc.vector.tensor_sub(wlo[sl][:], wt5[sl][0:4, :], whi16[sl][:])
                # wrep[(b,j), k] = per-band weight: bands 0-2 <- -2c,
                # 3-5 <- lo(-2c), 6 <- |c|^2, 7 <- 1 (two accumulated MMs
                # with constant selector matrices; no partition shifts)
                wrepP = smallps.tile([NB * J, K], F32, tag=f"small{sl}")
                nc.tensor.matmul(wrepP[:], bca[:], wt5[sl][:],
                                 start=True, stop=False)
                nc.tensor.matmul(wrepP[:], bcb[:], wlo[sl][:],
                                 start=False, stop=True)
                nc.scalar.copy(wrep[sl][:], wrepP[:])
                # wdiag16[(b,j), (k,j')] = wrep[(b,j), k] * 1[j==j']
                # (all-fp16 operands keep the DVE in 2x packed mode; the
                # fp16 rounding of wrep matches wdiag16's own rounding)
                wrep_b = bass.AP(
                    tensor=wrep[sl][:].tensor, offset=wrep[sl][:].offset,
                    ap=[wrep[sl][:].ap[0], [1, K], [0, J]])
                nc.vector.tensor_tensor(
                    wdiag16[sl][:].rearrange("p (k j) -> p k j", j=J),
                    diagk16[:].rearrange("p (k j) -> p k j", j=J),
                    wrep_b, mybir.AluOpType.mult)

            def batches_part(sl, x5b, xpb, carry=None, hook=None):
                # ---- main loop over super-batches. Seg matmuls trail their
                # mask by one super-batch, flushed in 8-MM chunks between
                # score bursts so the PE interleaves scores and seg work and
                # copies are never head-of-line blocked. The final super-
                # batch's chunks are RETURNED and flush inside the OTHER
                # image's batch loop (cross-image carry); `hook` emits that
                # image's fold/weights chain once the carry has drained. ----
                seg = segt[sl]
                flushq = list(carry) if carry else []

                if SEG_FP8:
                    def seg_chunk(mk, q, c):
                        def go():
                            mk8 = mk[:].bitcast(F8).rearrange(
                                "p (n two) -> p two n", two=2)
                            for t in range(c * 8, c * 8 + 8):
                                g = q * SDB + t
                                hf = g & 1
                                xp8 = xpb[:, g * XPW:(g + 1) * XPW].rearrange(
                                    "p (two c) -> p two c", two=2)
                                nc.tensor.matmul(
                                    seg[:, hf * FD:(hf + 1) * FD],
                                    xp8,
                                    mk8[:, :, t * FD:(t + 1) * FD],
                                    perf_mode=mybir.MatmulPerfMode.DoubleRow,
                                    start=(g == hf),
                                    stop=(g == groups - 2 + hf),
                                    skip_group_check=True)
                        return go
                else:
                    def seg_chunk(mk, q, c):
                        def go():
                            for t in range(c * 8, c * 8 + 8):
                                g = q * SDB + t
                                hf = g & 1
                                nc.tensor.matmul(
                                    seg[hf * CJ:(hf + 1) * CJ, :],
                                    xpb[:, g * CJ:(g + 1) * CJ],
                                    mk[:, t * FD:(t + 1) * FD],
                                    start=(g == hf),
                                    stop=(g == groups - 2 + hf),
                                    skip_group_check=True,
                                    tile_position=(0, hf * CJ))
                        return go

                for q in range(NSUP):
                    s16 = s16pool.tile([P, SDB * FD], F16, tag="s16")
                    if q == 1 and hook is not None:
                        hook()
                    for h in range(NFILL):
                        sp = bigpsum.tile([P, GBATCH * FD], F32, tag="big")
                        for t in range(GBATCH):
                            g = (q * NFILL + h) * GBATCH + t
                            nc.tensor.matmul(
                                sp[:, t * FD:(t + 1) * FD],
                                x5b[:, g * P:(g + 1) * P],
                                wdiag16[sl][:], start=True, stop=True)
                        if flushq:
                            flushq.pop(0)()
                        # ACT evacuates scores to fp16 SBUF (k-outer layout)
                        nc.scalar.copy(
                            s16[:, h * GBATCH * FD:(h + 1) * GBATCH * FD],
                            sp[:])
                    s4 = s16[:].rearrange("p (gb k j) -> p gb k j", k=K, j=J)
                    # 3-round pairwise min over k (all 2x packed-16 mode)
                    mv1 = mvpool.tile([P, SDB * 4 * J], F16, tag="mv1")
                    m1 = mv1[:].rearrange("p (gb k j) -> p gb k j", k=4, j=J)
                    nc.vector.tensor_tensor(
                        m1, s4[:, :, 0:4, :], s4[:, :, 4:8, :],
                        mybir.AluOpType.min)
                    mv2 = mvpool.tile([P, SDB * 2 * J], F16, tag="mv2")
                    m2 = mv2[:].rearrange("p (gb k j) -> p gb k j", k=2, j=J)
                    nc.vector.tensor_tensor(
                        m2, m1[:, :, 0:2, :], m1[:, :, 2:4, :],
                        mybir.AluOpType.min)
                    mv3 = mvpool.tile([P, SDB * J], F16, tag="mv3")
                    m3 = mv3[:].rearrange("p (gb j) -> p gb j", j=J)
                    nc.vector.tensor_tensor(
                        m3.unsqueeze(2), m2[:, :, 0:1, :], m2[:, :, 1:2, :],
                        mybir.AluOpType.min)
                    # mask = (s16 <= min) broadcast over k
                    mv_b = bass.AP(
                        tensor=mv3[:].tensor, offset=mv3[:].offset,
                        ap=[mv3[:].ap[0], [J, SDB], [0, K], [1, J]])
                    mk = maskpool.tile([P, SDB * FD], F16, tag="mk")
                    nc.vector.tensor_tensor(
                        mk[:].rearrange("p (gb k j) -> p gb k j", k=K, j=J),
                        s4, mv_b, mybir.AluOpType.is_le)
                    flushq.extend(seg_chunk(mk, q, c) for c in range(SDB // 8))
                return flushq

            def fold_update_part(sl):
                # ---- fold seg -> S[k, (r,g,b,count)] ----
                if SEG_FP8:
                    # sum the two parity chains, then one 64-row fold
                    # (only one PSUM operand allowed per instruction)
                    nc.scalar.copy(segsum[sl][:], segt[sl][:, 0:FD])
                    nc.vector.tensor_add(
                        segsum[sl][:], segsum[sl][:], segt[sl][:, FD:])
                    nc.vector.tensor_tensor(
                        prod[sl][:], segsum[sl][:], diagk[0:CJ, :],
                        mybir.AluOpType.mult)
                else:
                    nc.scalar.copy(segsb[sl][:], segt[sl][:])
                    nc.vector.tensor_tensor(
                        prod[sl][:], segsb[sl][:], diagk[:],
                        mybir.AluOpType.mult)
                nc.vector.tensor_reduce(
                    ext[sl][:],
                    prod[sl][:].rearrange("p (k j) -> p k j", j=J),
                    axis=mybir.AxisListType.X,
                    op=mybir.AluOpType.add)
                S = smallps.tile([K, 4], F32, tag=f"small{sl}")
                nc.tensor.matmul(S[:], ext[sl][:], csel[:],
                                 start=True, stop=True)

                # ---- centroid update ----
                # counts come back scaled by 1.5 in fp8 mode; threshold 0.5
                # keeps n=0 -> mean 0 (masked) and n>=1 exact in both modes.
                nc.vector.tensor_scalar_max(cntc[sl][:], S[:, 3:4], 0.5)
                nc.vector.reciprocal(recip[sl][:], cntc[sl][:])
                # per-partition scale rides the ACT ops (spares the DVE)
                nc.scalar.mul(cmean[sl][:], S[:, 0:D], recip[sl][:])
                nc.vector.tensor_scalar(
                    pos[sl][:], S[:, 3:4], 0.5, None,
                    op0=mybir.AluOpType.is_ge)
                pos_u = pos[sl][:].bitcast(mybir.dt.uint32)
                pos_b = bass.AP(
                    tensor=pos_u.tensor, offset=pos_u.offset,
                    ap=[pos_u.ap[0], [0, D]])
                nc.vector.copy_predicated(cent[sl][:], pos_b, cmean[sl][:])

            nc.sync.dma_start(out=cent[0][:], in_=c0_d[0])
            nc.sync.dma_start(out=cent[1][:], in_=c0_d[1])
            dma_image(0)
            dma_image(1)
            weights_part(0)              # pair 0's A weights

            for pair in range(n_img // 2):
                a, b = 2 * pair, 2 * pair + 1
                # prefetch next pair's A image; its x5/xpix slots are
                # unused by this pair. B's slot aliases image a's, so its
                # prefetch is issued after the trip loop below.
                if a + 2 < n_img:
                    dma_image(a + 2)

                xa, pa = x5t[a % NX5], xpixt[a % NXP]
                xb, pb = x5t[b % NX5], xpixt[b % NXP]

                # Fully symmetric software pipeline: each image's
                # fold/update/weights chain is emitted from a hook INSIDE
                # the other image's batch loop (after the carried seg
                # chunks drain), and each image's final super-batch of seg
                # matmuls flushes interleaved with the other image's score
                # bursts. On the last trip, A's result ships early and the
                # NEXT pair's A-centroids + weight chain are built under
                # B's final batch loop.
                def hook_b0():
                    weights_part(1)

                def hook_a():
                    fold_update_part(1)
                    weights_part(1)

                def hook_b():
                    fold_update_part(0)
                    weights_part(0)

                def hook_b_last():
                    fold_update_part(0)
                    nc.sync.dma_start(out=out_d[a], in_=cent[0][:])
                    if a + 2 < n_img:
                        nc.sync.dma_start(out=cent[0][:], in_=c0_d[a + 2])
                        weights_part(0)   # next pair's A weights

                carry = batches_part(0, xa, pa, hook=hook_b0)
                carry = batches_part(1, xb, pb, carry=carry, hook=hook_b)
                for t in range(1, iters):
                    carry = batches_part(0, xa, pa, carry=carry, hook=hook_a)
                    carry = batches_part(
                        1, xb, pb, carry=carry,
                        hook=hook_b if t < iters - 1 else hook_b_last)
                for chunk in carry:
                    chunk()
                fold_update_part(1)

                nc.sync.dma_start(out=out_d[b], in_=cent[1][:])
                if b + 2 < n_img:
                    nc.sync.dma_start(out=cent[1][:], in_=c0_d[b + 2])
                    dma_image(b + 2)

    nc.finalize()
    return nc


# ----------------------------------------------------------------------------
# host-side layouts


def host_layouts(pixels):
    """pixels [B, N, 3] f32 -> (x5 [B, 128, 16384] f16, xpix fp8/fp16).

    x5[(b,j), (g,p)]: bands 0-2 / 3-5 = x_rgb fp16 (hi/lo share data),
    band 6 = 1.0, band 7 = |x|^2 fp16, for pixel g*J*P + j*P + p.
    xpix fp8 mode: [p, (g, s, c, j)], s=0 zeros (pairs with the always-zero
    low mask byte), s=1 = fp8 pixel values; c in {r,g,b,1}.
    """
    b = pixels.shape[0]
    g = GROUPS
    v = pixels.reshape(b, g, J, P, D)
    rgb = np.ascontiguousarray(
        v.transpose(0, 4, 2, 1, 3).reshape(b, D * J, g * P)).astype(np.float16)
    xsq = (pixels.astype(np.float32) ** 2).sum(-1).astype(np.float16)
    xsqr = np.ascontiguousarray(
        xsq.reshape(b, g, J, P).transpose(0, 2, 1, 3).reshape(b, J, g * P))
    x5 = np.empty((b, NB * J, g * P), np.float16)
    x5[:, 0:48] = rgb
    x5[:, 48:96] = rgb
    x5[:, 96:112] = np.float16(1.0)
    x5[:, 112:128] = xsqr
    if SEG_FP8:
        xp = np.zeros((b, P, g, 2, 4, J), ml_dtypes.float8_e4m3)
        xp[..., 1, 0:3, :] = v.transpose(0, 3, 1, 4, 2).astype(
            ml_dtypes.float8_e4m3)  # b p g c j
        xp[..., 1, 3, :] = ml_dtypes.float8_e4m3(1.0)
        xpix = np.ascontiguousarray(xp.reshape(b, P, g * 2 * CJ))
    else:
        xp = np.empty((b, P, g, 4, J), np.float16)
        xp[..., 0:3, :] = v.transpose(0, 3, 1, 4, 2).astype(np.float16)
        xp[..., 3, :] = np.float16(1.0)
        xpix = np.ascontiguousarray(xp.reshape(b, P, g * CJ))
    return x5, xpix


def host_constants():
    diagk = np.zeros((NB * J, FD), np.float32)
    for bnd in range(NB):
        for j in range(J):
            for k in range(K):
                diagk[bnd * J + j, k * J + j] = 1.0
    if SEG_FP8:
        csel = np.zeros((CJ, 4), np.float32)
        for c in range(4):
            for j in range(J):
                csel[c * J + j, c] = 1.0
    else:
        csel = np.zeros((2 * CJ, 4), np.float32)
        for h in range(2):
            for c in range(4):
                for j in range(J):
                    csel[h * CJ + c * J + j, c] = 1.0
    # bca: wt5 rows (-2cx,-2cy,-2cz, cc, 1) -> bands (0,1,2, 6, 7)
    # bcb: wlo rows (lox,loy,loz, junk) -> bands (3,4,5, -)
    bca = np.zeros((5, NB * J), np.float32)
    bcb = np.zeros((4, NB * J), np.float32)
    for j in range(J):
        for r, bnd in enumerate((0, 1, 2, 6, 7)):
            bca[r, bnd * J + j] = 1.0
        for r, bnd in enumerate((3, 4, 5)):
            bcb[r, bnd * J + j] = 1.0
    ident = np.eye(K, dtype=np.float32)
    return diagk, csel, bca, bcb, ident


_NC_CACHE = {}
TRACE = False
LAST_RESULTS = None


def _get_nc(n_img, iters, groups):
    key = (n_img, iters, groups)
    if key not in _NC_CACHE:
        _NC_CACHE[key] = build_kernel(n_img, iters, groups)
    return _NC_CACHE[key]


def kernel(inputs: np.ndarray) -> np.ndarray:
    x = np.ascontiguousarray(np.asarray(inputs, dtype=np.float32))
    assert x.shape == (B, H, W, D), x.shape
    pixels = x.reshape(B, N, D)

    perm8 = np.array(PERM8, dtype=np.int64)
    cent0 = np.take_along_axis(
        pixels, perm8[:, :, None].repeat(D, axis=2), axis=1
    ).astype(np.float32)

    x5, xpix = host_layouts(pixels)
    diagk, csel, bca, bcb, ident = host_constants()
    nc = _get_nc(IMG_PER_CORE, ITERS, GROUPS)

    in_maps = []
    for c in range(NCORES):
        sl = slice(c * IMG_PER_CORE, (c + 1) * IMG_PER_CORE)
        in_maps.append({
            "x5": np.ascontiguousarray(x5[sl]),
            "xpix": np.ascontiguousarray(xpix[sl]),
            "cent0": np.ascontiguousarray(cent0[sl]),
            "diagk": diagk,
            "csel": csel,
            "bca": bca,
            "bcb": bcb,
            "ident": ident,
        })

    global LAST_RESULTS
    try:
        res = run_bass_kernel_spmd(nc, in_maps, core_ids=list(range(NCORES)),
                                   trace=TRACE)
    except Exception:
        if not TRACE:
            raise
        res = run_bass_kernel_spmd(nc, in_maps, core_ids=list(range(NCORES)))
    LAST_RESULTS = res
    outs = [r["cent_out"].reshape(IMG_PER_CORE, K * D) for r in res.results]
    return np.concatenate(outs, axis=0).astype(np.float32)


if __name__ == "__main__":
    rs = np.random.RandomState(0)
    x = rs.random_sample((B, H, W, D)).astype(np.float32)
    out = kernel(inputs=x)
    print("out shape", out.shape, out.dtype)
    print(out[0])
